# revision 16
# baseline (speedup 1.0000x reference)
"""AdaptiveFilterAttention on 8 TRN2 NeuronCores.

Sharding: 32 (batch, head) pairs -> 8 cores; core c handles batch c//4,
local head group c%4 (4 heads). Per core: QKV projections for its 256
output dims, per-head attention with exp(-alpha*|i-j|) decay folded in
via rank-1 row scalings of q/k (decay factors exp(+-alpha*t) multiply q
and k rows; diagonal-crossing tiles get a fixup multiply), softmax
without max-subtraction (scores are bounded small), attn@v with an
appended ones-column producing the softmax denominators for free, and a
row-parallel output projection producing a partial (T, D) result summed
on the host across the 4 cores of each batch.
"""
import os
import sys

import numpy as np
import ml_dtypes

sys.path.insert(0, "/opt/trn_rl_repo")

import concourse.bass as bass  # noqa: E402
import concourse.mybir as mybir  # noqa: E402
import concourse.tile as tile  # noqa: E402
from concourse import bacc  # noqa: E402
from concourse.bass_utils import run_bass_kernel_spmd  # noqa: E402

BF16 = mybir.dt.bfloat16
F32 = mybir.dt.float32
P = 128
B, T, D = 2, 2048, 1024
H, HD = 16, 64
HPC = 4            # heads per core
MPC = HD * HPC // P  # 2: partition-tiles of this core's 256 proj dims
NQ = 512           # q free-tile
NT = T // NQ       # 4
KBLK = T // P      # 16 k-blocks of 128
NCORES = 8
DT_CONST = 1.0


def _patch_walrus_ldw_opt():
    """Rewrite --enable-ldw-opt=false -> true in walrus invocation."""
    import concourse.bass_utils as bu
    if getattr(bu, "_ldw_patched", False):
        return
    orig = bu.run_command

    def run_command_ldw(cmd, *a, **kw):
        if isinstance(cmd, list):
            cmd = ["--enable-ldw-opt=true" if c == "--enable-ldw-opt=false"
                   else c for c in cmd]
        return orig(cmd, *a, **kw)

    bu.run_command = run_command_ldw
    bu._ldw_patched = True



LAST_EXEC_NS = None
LAST_RESULT = None
_GRAPH_CACHE = {}


def _build(kp):
    """Build the per-core Bass graph. kp = number of 128-row contraction
    tiles in the projections (8 without bias row, 9 with)."""
    nc = bacc.Bacc(None, target_bir_lowering=False)

    xT_ext = nc.declare_dram_parameter("xT", [kp * P, T], BF16, isOutput=False)
    wq_ext = nc.declare_dram_parameter("wq", [kp * P, 256], BF16, isOutput=False)
    wk_ext = nc.declare_dram_parameter("wk", [kp * P, 256], BF16, isOutput=False)
    wv_ext = nc.declare_dram_parameter("wv", [kp * P, 256], BF16, isOutput=False)
    wo_ext = nc.declare_dram_parameter("wo", [256, D], BF16, isOutput=False)
    rqlo_ext = nc.declare_dram_parameter("rqlo", [P, T], F32, isOutput=False)
    rqhi_ext = nc.declare_dram_parameter("rqhi", [P, T], F32, isOutput=False)
    rklo_ext = nc.declare_dram_parameter("rklo", [P, T], F32, isOutput=False)
    rkhi_ext = nc.declare_dram_parameter("rkhi", [P, T], F32, isOutput=False)
    corr_ext = nc.declare_dram_parameter("corr", [P, T], F32, isOutput=False)
    out_ext = nc.declare_dram_parameter("out", [D, T], BF16, isOutput=True)

    with tile.TileContext(nc) as tc:
        with tc.tile_pool(name="consts", bufs=1) as consts, \
             tc.tile_pool(name="vars", bufs=1) as vars_p, \
             tc.tile_pool(name="dram", bufs=8, space="DRAM") as dram_p:

            xt_sb = consts.tile([P, kp, T], BF16)
            wq_sb = consts.tile([P, kp, 256], BF16)
            wk_sb = consts.tile([P, kp, 256], BF16)
            wv_sb = consts.tile([P, kp, 256], BF16)
            wo_sb = consts.tile([P, 2, D], BF16)
            rqlo = consts.tile([P, T], F32)
            rqhi = consts.tile([P, T], F32)
            rklo = consts.tile([P, T], F32)
            rkhi = consts.tile([P, T], F32)
            corr_sb = consts.tile([P, T], F32)

            for kt in range(kp):
                nc.sync.dma_start(xt_sb[:, kt, :], xT_ext[kt * P:(kt + 1) * P, :])
                nc.sync.dma_start(wq_sb[:, kt, :], wq_ext[kt * P:(kt + 1) * P, :])
                nc.sync.dma_start(wk_sb[:, kt, :], wk_ext[kt * P:(kt + 1) * P, :])
                nc.sync.dma_start(wv_sb[:, kt, :], wv_ext[kt * P:(kt + 1) * P, :])
            for kt2 in range(2):
                nc.sync.dma_start(wo_sb[:, kt2, :], wo_ext[kt2 * P:(kt2 + 1) * P, :])
            nc.sync.dma_start(rqlo[:], rqlo_ext[:])
            nc.sync.dma_start(rqhi[:], rqhi_ext[:])
            nc.sync.dma_start(rklo[:], rklo_ext[:])
            nc.sync.dma_start(rkhi[:], rkhi_ext[:])
            nc.sync.dma_start(corr_sb[:], corr_ext[:])

            # persistent per-core tensors
            q_lo = vars_p.tile([P, MPC, T], BF16)
            q_hi = vars_p.tile([P, MPC, T], BF16)
            k_lo = vars_p.tile([P, MPC, T], BF16)
            k_hi = vars_p.tile([P, MPC, T], BF16)
            v_sb = vars_p.tile([P, KBLK, HPC, HD + 1], BF16)
            o_all = vars_p.tile([P, MPC, T], BF16)

            nc.vector.memset(v_sb[:, :, :, HD:HD + 1], 1.0)

            # ---- Stage A: projections -------------------------------------
            with tc.tile_pool(name="psA", bufs=4, space="PSUM") as psA, \
                 tc.tile_pool(name="psV", bufs=2, space="PSUM") as psV:
                for w_t, lo_r, hi_r, lo_d, hi_d in (
                    (wq_sb, rqlo, rqhi, q_lo, q_hi),
                    (wk_sb, rklo, rkhi, k_lo, k_hi),
                ):
                    for mt in range(MPC):
                        pts = [psA.tile([P, NQ], F32, tag="projps", name=f"pt{_n}")
                               for _n in range(NT)]
                        for kt in range(kp):
                            for nt in range(NT):
                                nc.tensor.matmul(
                                    pts[nt],
                                    w_t[:, kt, mt * P:(mt + 1) * P],
                                    xt_sb[:, kt, nt * NQ:(nt + 1) * NQ],
                                    start=(kt == 0), stop=(kt == kp - 1),
                                )
                        for nt in range(NT):
                            sl = slice(nt * NQ, (nt + 1) * NQ)
                            nc.vector.tensor_tensor(
                                lo_d[:, mt, sl], pts[nt], lo_r[:, sl],
                                mybir.AluOpType.mult)
                            nc.vector.tensor_tensor(
                                hi_d[:, mt, sl], pts[nt], hi_r[:, sl],
                                mybir.AluOpType.mult)
                # v projection: x^T-stationary so v lands [token, dim]
                for mt in range(KBLK):
                    pv = psV.tile([P, 256], F32, tag="vps")
                    for kt in range(kp):
                        nc.tensor.matmul(
                            pv,
                            xt_sb[:, kt, mt * P:(mt + 1) * P],
                            wv_sb[:, kt, :],
                            start=(kt == 0), stop=(kt == kp - 1),
                        )
                    nc.vector.tensor_copy(
                        v_sb[:, mt, :, 0:HD],
                        pv.rearrange("p (h d) -> p h d", h=HPC),
                    )

            # ---- Stage B: attention ---------------------------------------
            # Far-from-diagonal k-blocks: decay ~ 0 so E = exp(~0) ~ 1.
            # Their contribution to O (incl. the softmax denominator via the
            # ones column of v') is a q-independent vector: quarter-aligned
            # prefix sums of per-k-block v' column sums, injected as an ACT
            # bias during the accumulator merge. Near blocks: scores as
            # row-packed K=64 pairs (two heads concurrently), exp on ACT,
            # then attn@v as row-packed K=64 pairs (even/odd token halves
            # into separate PSUM accumulators, merged during normalization).
            FAR_TAU = 512

            def far(qt, kb):
                lo = 512 * qt - 128 * kb - 127   # min Delta when q above k
                hi = 128 * kb - 512 * qt - 511   # min -Delta when k above q
                return lo >= FAR_TAU or hi >= FAR_TAU

            def near_kbs(qt):
                return [kb for kb in range(KBLK) if not far(qt, kb)]

            def cls_of(qt, kb):
                c = kb // 4
                if c < qt:
                    return "lo"
                if c == qt:
                    return "cross"
                return "hi"

            # farsum combos per head: cols 0..3 = quarter sums Q0..Q3 of
            # vcolsum, col 4 = Q0+Q1, col 5 = Q2+Q3.
            # far set: qt0 -> Q2+Q3 (col5), qt1 -> Q3 (col3),
            #          qt2 -> Q0 (col0), qt3 -> Q0+Q1 (col4)
            FARCOL = {0: 5, 1: 3, 2: 0, 3: 4}
            ones_col = vars_p.tile([P, 1], BF16)
            nc.vector.memset(ones_col[:], 1.0)
            qsum = vars_p.tile([P, HPC, 6], F32)
            with tc.tile_pool(name="qpool", bufs=2, space="PSUM") as qpool:
                for h in range(HPC):
                    pf = qpool.tile([P, 4], F32, tag="pf")
                    for g in range(4):
                        for kk in range(4):
                            nc.tensor.matmul(
                                pf[0:HD + 1, g:g + 1],
                                v_sb[:, 4 * g + kk, h, :],
                                ones_col[:],
                                start=(kk == 0), stop=(kk == 3),
                            )
                    nc.vector.tensor_copy(qsum[0:HD + 1, h, 0:4],
                                          pf[0:HD + 1, :])
                    nc.vector.tensor_tensor(
                        qsum[0:HD + 1, h, 4:5], qsum[0:HD + 1, h, 0:1],
                        qsum[0:HD + 1, h, 1:2], mybir.AluOpType.add)
                    nc.vector.tensor_tensor(
                        qsum[0:HD + 1, h, 5:6], qsum[0:HD + 1, h, 2:3],
                        qsum[0:HD + 1, h, 3:4], mybir.AluOpType.add)

            with tc.tile_pool(name="spool", bufs=2, space="PSUM") as spool, \
                 tc.tile_pool(name="opool", bufs=4, space="PSUM") as opool, \
                 tc.tile_pool(name="epool", bufs=6) as epool, \
                 tc.tile_pool(name="npool", bufs=4) as npool:
                for pg in range(MPC):
                    for qt in range(NT):
                        qsl = slice(qt * NQ, (qt + 1) * NQ)
                        nears = near_kbs(qt)
                        kpairs = [(nears[2 * i], nears[2 * i + 1])
                                  for i in range(len(nears) // 2)]
                        ope = [opool.tile([P, NQ], F32, tag="ops",
                                          name=f"ope{_n}") for _n in range(2)]
                        opo = [opool.tile([P, NQ], F32, tag="ops",
                                          name=f"opo{_n}") for _n in range(2)]
                        pend = []

                        def emit_vmms(pkp, paps):
                            first_p = pkp[0] == nears[0]
                            last_p = pkp[1] == nears[-1]
                            for x in range(2):
                                for j, pkb in enumerate(pkp):
                                    jsl = slice(j * NQ, (j + 1) * NQ)
                                    first = first_p and j == 0
                                    last = last_p and j == 1
                                    nc.tensor.matmul(
                                        ope[x][0:HD + 1, :],
                                        v_sb[0:HD, pkb, 2 * pg + x, :],
                                        paps[x][0:HD, jsl],
                                        start=first, stop=last,
                                    )
                                    nc.tensor.matmul(
                                        opo[x][0:HD + 1, :],
                                        v_sb[HD:P, pkb, 2 * pg + x, :],
                                        paps[x][HD:P, jsl],
                                        start=first, stop=last,
                                    )

                        for kp in kpairs:
                            pss = [spool.tile([P, 2 * NQ], F32, tag="spool",
                                              name=f"ps{_n}")
                                   for _n in range(2)]
                            for j, kb in enumerate(kp):
                                cls = cls_of(qt, kb)
                                qvar, kvar = ((q_lo, k_lo) if cls != "hi"
                                              else (q_hi, k_hi))
                                ksl = slice(kb * P, (kb + 1) * P)
                                jsl = slice(j * NQ, (j + 1) * NQ)
                                for x in range(2):
                                    psl = slice(x * HD, (x + 1) * HD)
                                    nc.tensor.matmul(
                                        pss[x][:, jsl], kvar[psl, pg, ksl],
                                        qvar[psl, pg, qsl],
                                        start=True, stop=True,
                                    )
                            e_aps = [None, None]
                            both_cross = all(cls_of(qt, kb) == "cross"
                                             for kb in kp)
                            for x in range(2):
                                if both_cross:
                                    off = (kp[0] - 4 * qt) * NQ
                                    nc.vector.tensor_tensor(
                                        pss[x][:], pss[x][:],
                                        corr_sb[:, off:off + 2 * NQ],
                                        mybir.AluOpType.mult)
                                elif any(cls_of(qt, kb) == "cross"
                                         for kb in kp):
                                    for j, kb in enumerate(kp):
                                        if cls_of(qt, kb) != "cross":
                                            continue
                                        jsl = slice(j * NQ, (j + 1) * NQ)
                                        off = (kb - 4 * qt) * NQ
                                        nc.vector.tensor_tensor(
                                            pss[x][:, jsl], pss[x][:, jsl],
                                            corr_sb[:, off:off + NQ],
                                            mybir.AluOpType.mult)
                                e_t = epool.tile([P, 2 * NQ], BF16, tag="e")
                                nc.scalar.activation(
                                    e_t[:], pss[x][:],
                                    mybir.ActivationFunctionType.Exp)
                                e_aps[x] = e_t
                            if len(pend) >= 1:
                                emit_vmms(*pend.pop(0))
                            for _d in range(3):
                                nc.tensor.ldweights(q_lo[:, 0, 0:P])
                            pend.append((kp, e_aps))
                        for pe_ in pend:
                            emit_vmms(*pe_)

                        # merge accumulators + far injection + normalize
                        for x in range(2):
                            col = FARCOL[qt]
                            stg = npool.tile([P, NQ], F32, tag="stg")
                            nc.scalar.add(
                                stg[0:HD + 1, :], opo[x][0:HD + 1, :],
                                qsum[0:HD + 1, 2 * pg + x, col:col + 1])
                            nc.vector.tensor_tensor(
                                stg[0:HD + 1, :], ope[x][0:HD + 1, :],
                                stg[0:HD + 1, :], mybir.AluOpType.add)
                            dsum = dram_p.tile([1, NQ], F32, tag="dsum")
                            nc.sync.dma_start(dsum[:], stg[HD:HD + 1, :])
                            srep = npool.tile([HD, NQ], F32, tag="srep")
                            nc.sync.dma_start(
                                srep[:], dsum[:].to_broadcast((HD, NQ)))
                            rrep = npool.tile([HD, NQ], F32, tag="rrep")
                            nc.vector.reciprocal_approx_fast(rrep[:], srep[:])
                            if x == 0:
                                nc.vector.tensor_tensor(
                                    o_all[0:HD, pg, qsl], stg[0:HD, :],
                                    rrep[:], mybir.AluOpType.mult)
                            else:
                                ob = npool.tile([HD, NQ], BF16, tag="ob")
                                nc.vector.tensor_tensor(
                                    ob[:], stg[0:HD, :], rrep[:],
                                    mybir.AluOpType.mult)
                                nc.sync.dma_start(o_all[HD:P, pg, qsl],
                                                  ob[:])

            # ---- Stage C: output projection -------------------------------
            with tc.tile_pool(name="cpool", bufs=2, space="PSUM") as cpool, \
                 tc.tile_pool(name="fpool", bufs=2) as fpool:
                for mt in range(D // P):
                    pc = cpool.tile([P, T], F32, tag="cps")
                    for kt2 in range(2):
                        for nt in range(NT):
                            nc.tensor.matmul(
                                pc[:, nt * NQ:(nt + 1) * NQ],
                                wo_sb[:, kt2, mt * P:(mt + 1) * P],
                                o_all[:, kt2, nt * NQ:(nt + 1) * NQ],
                                start=(kt2 == 0), stop=(kt2 == 1),
                            )
                    fo = fpool.tile([P, T], BF16, tag="fo")
                    if mt % 2 == 0:
                        nc.vector.tensor_copy(fo[:], pc[:])
                    else:
                        nc.scalar.copy(fo[:], pc[:])
                    nc.sync.dma_start(out_ext[mt * P:(mt + 1) * P, :], fo[:])

    nc.finalize()
    return nc


def _get_graph(kp):
    if kp not in _GRAPH_CACHE:
        _GRAPH_CACHE[kp] = _build(kp)
    return _GRAPH_CACHE[kp]


def _install_trace_hooks():
    import types
    import antenv
    if "antenv.axon_hooks" not in sys.modules:
        hooks = types.ModuleType("antenv.axon_hooks")
        hooks._hook = None
        hooks.set_axon_ntff_profile_hook = lambda h: setattr(hooks, "_hook", h)
        hooks.get_axon_ntff_profile_hook = lambda: hooks._hook
        sys.modules["antenv.axon_hooks"] = hooks
        antenv.axon_hooks = hooks
    if sys.modules["antenv.axon_hooks"]._hook is None:
        if "/root/.axon_site" not in sys.path:
            sys.path.insert(0, "/root/.axon_site")
        from trn_agent_boot.trn_boot import _ntff_profile_via_ctypes
        sys.modules["antenv.axon_hooks"].set_axon_ntff_profile_hook(
            _ntff_profile_via_ctypes("/opt/axon/libaxon_pjrt.so"))


def kernel(x, Wq, bq, Wk, bk, Wv, bv, Wo, bo, alpha):
    global LAST_EXEC_NS, LAST_RESULT
    x = np.asarray(x, dtype=np.float32)
    Wq = np.asarray(Wq, dtype=np.float32)
    Wk = np.asarray(Wk, dtype=np.float32)
    Wv = np.asarray(Wv, dtype=np.float32)
    Wo = np.asarray(Wo, dtype=np.float32)
    bq = np.asarray(bq, dtype=np.float32)
    bk = np.asarray(bk, dtype=np.float32)
    bv = np.asarray(bv, dtype=np.float32)
    bo = np.asarray(bo, dtype=np.float32)
    alpha = float(np.asarray(alpha))
    a_eff = alpha * DT_CONST
    scale = HD ** -0.5

    has_bias = bool(np.any(bq) or np.any(bk) or np.any(bv))
    kp = 9 if has_bias else 8
    nc = _get_graph(kp)

    t_idx = np.arange(T, dtype=np.float64)
    e_neg = np.exp(-a_eff * t_idx)
    e_pos = np.exp(+a_eff * t_idx)
    rqlo = np.tile((scale * e_neg).astype(np.float32), (P, 1))
    rqhi = np.tile((scale * e_pos).astype(np.float32), (P, 1))
    rklo = np.tile(e_pos.astype(np.float32), (P, 1))
    rkhi = np.tile(e_neg.astype(np.float32), (P, 1))

    # corr[kk, o*512+qq] = 1 if d>=0 else exp(2*a_eff*d), d = qq-kk-128*o
    kk = np.arange(P)[:, None]
    qq = np.arange(NQ)[None, :]
    corr = np.empty((P, T), dtype=np.float32)
    for o in range(4):
        d = qq - kk - P * o
        corr[:, o * NQ:(o + 1) * NQ] = np.where(
            d >= 0, 1.0, np.exp(2.0 * a_eff * d))

    def wslice(W, b, g):
        ws = W[256 * g:256 * g + 256, :].T.astype(np.float64)
        if has_bias:
            ws = np.vstack([ws, b[256 * g:256 * g + 256][None, :],
                            np.zeros((kp * P - D - 1, 256))])
        return np.ascontiguousarray(ws).astype(ml_dtypes.bfloat16)

    in_maps = []
    for core in range(NCORES):
        b_idx, g = core // 4, core % 4
        xT = x[b_idx].T.astype(np.float64)
        if has_bias:
            xT = np.vstack([xT, np.ones((1, T)), np.zeros((kp * P - D - 1, T))])
        in_maps.append({
            "xT": np.ascontiguousarray(xT).astype(ml_dtypes.bfloat16),
            "wq": wslice(Wq, bq, g),
            "wk": wslice(Wk, bk, g),
            "wv": wslice(Wv, bv, g),
            "wo": np.ascontiguousarray(
                Wo[:, 256 * g:256 * g + 256].T).astype(ml_dtypes.bfloat16),
            "rqlo": rqlo, "rqhi": rqhi, "rklo": rklo, "rkhi": rkhi,
            "corr": corr,
        })

    trace = bool(os.environ.get("BASS_KERNEL_TRACE"))
    if trace:
        _install_trace_hooks()
    res = run_bass_kernel_spmd(nc, in_maps, core_ids=list(range(NCORES)),
                               trace=trace)
    LAST_EXEC_NS = res.exec_time_ns
    LAST_RESULT = res

    out = np.empty((B, T, D), dtype=np.float32)
    for b_idx in range(B):
        acc = np.zeros((D, T), dtype=np.float32)
        for g in range(4):
            acc += np.asarray(res.results[b_idx * 4 + g]["out"],
                              dtype=np.float32)
        out[b_idx] = acc.T + bo[None, :]
    return out


# revision 17
# speedup vs baseline: 1.2048x; 1.2048x over previous
"""AdaptiveFilterAttention on 8 TRN2 NeuronCores.

Sharding: 32 (batch, head) pairs -> 8 cores; core c handles batch c//4,
local head group c%4 (4 heads). Per core: QKV projections for its 256
output dims, per-head attention with exp(-alpha*|i-j|) decay folded in
via rank-1 row scalings of q/k (decay factors exp(+-alpha*t) multiply q
and k rows; diagonal-crossing tiles get a fixup multiply), softmax
without max-subtraction (scores are bounded small), attn@v with an
appended ones-column producing the softmax denominators for free, and a
row-parallel output projection producing a partial (T, D) result summed
on the host across the 4 cores of each batch.
"""
import os
import sys

import numpy as np
import ml_dtypes

sys.path.insert(0, "/opt/trn_rl_repo")

import concourse.bass as bass  # noqa: E402
import concourse.mybir as mybir  # noqa: E402
import concourse.tile as tile  # noqa: E402
from concourse import bacc  # noqa: E402
from concourse.bass_utils import run_bass_kernel_spmd  # noqa: E402

BF16 = mybir.dt.bfloat16
F32 = mybir.dt.float32
P = 128
B, T, D = 2, 2048, 1024
H, HD = 16, 64
HPC = 4            # heads per core
MPC = HD * HPC // P  # 2: partition-tiles of this core's 256 proj dims
NQ = 512           # q free-tile
NT = T // NQ       # 4
KBLK = T // P      # 16 k-blocks of 128
NCORES = 8
DT_CONST = 1.0


def _patch_walrus_ldw_opt():
    """Rewrite --enable-ldw-opt=false -> true in walrus invocation."""
    import concourse.bass_utils as bu
    if getattr(bu, "_ldw_patched", False):
        return
    orig = bu.run_command

    def run_command_ldw(cmd, *a, **kw):
        if isinstance(cmd, list):
            cmd = ["--enable-ldw-opt=true" if c == "--enable-ldw-opt=false"
                   else c for c in cmd]
        return orig(cmd, *a, **kw)

    bu.run_command = run_command_ldw
    bu._ldw_patched = True



LAST_EXEC_NS = None
LAST_RESULT = None
_GRAPH_CACHE = {}


def _build(kp):
    """Build the per-core Bass graph. kp = number of 128-row contraction
    tiles in the projections (8 without bias row, 9 with)."""
    nc = bacc.Bacc(None, target_bir_lowering=False)

    xT_ext = nc.declare_dram_parameter("xT", [kp * P, T], BF16, isOutput=False)
    wq_ext = nc.declare_dram_parameter("wq", [kp * P, 256], BF16, isOutput=False)
    wk_ext = nc.declare_dram_parameter("wk", [kp * P, 256], BF16, isOutput=False)
    wv_ext = nc.declare_dram_parameter("wv", [kp * P, 256], BF16, isOutput=False)
    wo_ext = nc.declare_dram_parameter("wo", [256, D], BF16, isOutput=False)
    rqlo_ext = nc.declare_dram_parameter("rqlo", [P, T], F32, isOutput=False)
    rqhi_ext = nc.declare_dram_parameter("rqhi", [P, T], F32, isOutput=False)
    rklo_ext = nc.declare_dram_parameter("rklo", [P, T], F32, isOutput=False)
    rkhi_ext = nc.declare_dram_parameter("rkhi", [P, T], F32, isOutput=False)
    corr_ext = nc.declare_dram_parameter("corr", [P, T], F32, isOutput=False)
    out_ext = nc.declare_dram_parameter("out", [D, T], BF16, isOutput=True)

    with tile.TileContext(nc) as tc:
        with tc.tile_pool(name="consts", bufs=1) as consts, \
             tc.tile_pool(name="vars", bufs=1) as vars_p, \
             tc.tile_pool(name="dram", bufs=8, space="DRAM") as dram_p:

            xt_sb = consts.tile([P, kp, T], BF16)
            wq_sb = consts.tile([P, kp, 256], BF16)
            wk_sb = consts.tile([P, kp, 256], BF16)
            wv_sb = consts.tile([P, kp, 256], BF16)
            wo_sb = consts.tile([P, 2, D], BF16)
            rqlo = consts.tile([P, T], F32)
            rqhi = consts.tile([P, T], F32)
            rklo = consts.tile([P, T], F32)
            rkhi = consts.tile([P, T], F32)
            corr_sb = consts.tile([P, T], F32)

            for kt in range(kp):
                nc.sync.dma_start(xt_sb[:, kt, :], xT_ext[kt * P:(kt + 1) * P, :])
                nc.sync.dma_start(wq_sb[:, kt, :], wq_ext[kt * P:(kt + 1) * P, :])
                nc.sync.dma_start(wk_sb[:, kt, :], wk_ext[kt * P:(kt + 1) * P, :])
                nc.sync.dma_start(wv_sb[:, kt, :], wv_ext[kt * P:(kt + 1) * P, :])
            for kt2 in range(2):
                nc.sync.dma_start(wo_sb[:, kt2, :], wo_ext[kt2 * P:(kt2 + 1) * P, :])
            nc.sync.dma_start(rqlo[:], rqlo_ext[:])
            nc.sync.dma_start(rqhi[:], rqhi_ext[:])
            nc.sync.dma_start(rklo[:], rklo_ext[:])
            nc.sync.dma_start(rkhi[:], rkhi_ext[:])
            nc.sync.dma_start(corr_sb[:], corr_ext[:])

            # persistent per-core tensors
            q_lo = vars_p.tile([P, MPC, T], BF16)
            q_hi = vars_p.tile([P, MPC, T], BF16)
            k_lo = vars_p.tile([P, MPC, T], BF16)
            k_hi = vars_p.tile([P, MPC, T], BF16)
            v_sb = vars_p.tile([P, KBLK, HPC, HD + 1], BF16)
            o_all = vars_p.tile([P, MPC, T], BF16)

            nc.vector.memset(v_sb[:, :, :, HD:HD + 1], 1.0)

            # ---- Stage A: projections -------------------------------------
            with tc.tile_pool(name="psA", bufs=4, space="PSUM") as psA, \
                 tc.tile_pool(name="psV", bufs=2, space="PSUM") as psV:
                for w_t, lo_r, hi_r, lo_d, hi_d in (
                    (wq_sb, rqlo, rqhi, q_lo, q_hi),
                    (wk_sb, rklo, rkhi, k_lo, k_hi),
                ):
                    for mt in range(MPC):
                        pts = [psA.tile([P, NQ], F32, tag="projps", name=f"pt{_n}")
                               for _n in range(NT)]
                        for kt in range(kp):
                            for nt in range(NT):
                                nc.tensor.matmul(
                                    pts[nt],
                                    w_t[:, kt, mt * P:(mt + 1) * P],
                                    xt_sb[:, kt, nt * NQ:(nt + 1) * NQ],
                                    start=(kt == 0), stop=(kt == kp - 1),
                                )
                        for nt in range(NT):
                            sl = slice(nt * NQ, (nt + 1) * NQ)
                            nc.vector.tensor_tensor(
                                lo_d[:, mt, sl], pts[nt], lo_r[:, sl],
                                mybir.AluOpType.mult)
                            nc.vector.tensor_tensor(
                                hi_d[:, mt, sl], pts[nt], hi_r[:, sl],
                                mybir.AluOpType.mult)
                # v projection: x^T-stationary so v lands [token, dim]
                for mt in range(KBLK):
                    pv = psV.tile([P, 256], F32, tag="vps")
                    for kt in range(kp):
                        nc.tensor.matmul(
                            pv,
                            xt_sb[:, kt, mt * P:(mt + 1) * P],
                            wv_sb[:, kt, :],
                            start=(kt == 0), stop=(kt == kp - 1),
                        )
                    nc.vector.tensor_copy(
                        v_sb[:, mt, :, 0:HD],
                        pv.rearrange("p (h d) -> p h d", h=HPC),
                    )

            # ---- Stage B: attention ---------------------------------------
            # Far-from-diagonal k-blocks: decay ~ 0 so E = exp(~0) ~ 1.
            # Their contribution to O (incl. the softmax denominator via the
            # ones column of v') is a q-independent vector: quarter-aligned
            # prefix sums of per-k-block v' column sums, injected as an ACT
            # bias during the accumulator merge. Near blocks: scores as
            # row-packed K=64 pairs (two heads concurrently), exp on ACT,
            # then attn@v as row-packed K=64 pairs (even/odd token halves
            # into separate PSUM accumulators, merged during normalization).
            FAR_TAU = 512

            def far(qt, kb):
                lo = 512 * qt - 128 * kb - 127   # min Delta when q above k
                hi = 128 * kb - 512 * qt - 511   # min -Delta when k above q
                return lo >= FAR_TAU or hi >= FAR_TAU

            def near_kbs(qt):
                return [kb for kb in range(KBLK) if not far(qt, kb)]

            def cls_of(qt, kb):
                c = kb // 4
                if c < qt:
                    return "lo"
                if c == qt:
                    return "cross"
                return "hi"

            # farsum combos per head: cols 0..3 = quarter sums Q0..Q3 of
            # vcolsum, col 4 = Q0+Q1, col 5 = Q2+Q3.
            # far set: qt0 -> Q2+Q3 (col5), qt1 -> Q3 (col3),
            #          qt2 -> Q0 (col0), qt3 -> Q0+Q1 (col4)
            FARCOL = {0: 5, 1: 3, 2: 0, 3: 4}
            ones_col = vars_p.tile([P, 1], BF16)
            nc.vector.memset(ones_col[:], 1.0)
            qsum = vars_p.tile([P, HPC, 6], F32)
            with tc.tile_pool(name="qpool", bufs=2, space="PSUM") as qpool:
                for h in range(HPC):
                    pf = qpool.tile([P, 4], F32, tag="pf")
                    for g in range(4):
                        for kk in range(4):
                            nc.tensor.matmul(
                                pf[0:HD + 1, g:g + 1],
                                v_sb[:, 4 * g + kk, h, :],
                                ones_col[:],
                                start=(kk == 0), stop=(kk == 3),
                            )
                    nc.vector.tensor_copy(qsum[0:HD + 1, h, 0:4],
                                          pf[0:HD + 1, :])
                    nc.vector.tensor_tensor(
                        qsum[0:HD + 1, h, 4:5], qsum[0:HD + 1, h, 0:1],
                        qsum[0:HD + 1, h, 1:2], mybir.AluOpType.add)
                    nc.vector.tensor_tensor(
                        qsum[0:HD + 1, h, 5:6], qsum[0:HD + 1, h, 2:3],
                        qsum[0:HD + 1, h, 3:4], mybir.AluOpType.add)

            with tc.tile_pool(name="spool", bufs=2, space="PSUM") as spool, \
                 tc.tile_pool(name="opool", bufs=4, space="PSUM") as opool, \
                 tc.tile_pool(name="epool", bufs=4) as epool, \
                 tc.tile_pool(name="npool", bufs=4) as npool:
                for pg in range(MPC):
                    for qt in range(NT):
                        qsl = slice(qt * NQ, (qt + 1) * NQ)
                        nears = near_kbs(qt)
                        kpairs = [(nears[2 * i], nears[2 * i + 1])
                                  for i in range(len(nears) // 2)]
                        ope = [opool.tile([P, NQ], F32, tag="ops",
                                          name=f"ope{_n}") for _n in range(2)]
                        opo = [opool.tile([P, NQ], F32, tag="ops",
                                          name=f"opo{_n}") for _n in range(2)]
                        pend = []

                        def emit_vmms(pkp, paps):
                            first_p = pkp[0] == nears[0]
                            last_p = pkp[1] == nears[-1]
                            for x in range(2):
                                for j, pkb in enumerate(pkp):
                                    jsl = slice(j * NQ, (j + 1) * NQ)
                                    first = first_p and j == 0
                                    last = last_p and j == 1
                                    nc.tensor.matmul(
                                        ope[x][0:HD + 1, :],
                                        v_sb[0:HD, pkb, 2 * pg + x, :],
                                        paps[x][0:HD, jsl],
                                        start=first, stop=last,
                                    )
                                    nc.tensor.matmul(
                                        opo[x][0:HD + 1, :],
                                        v_sb[HD:P, pkb, 2 * pg + x, :],
                                        paps[x][HD:P, jsl],
                                        start=first, stop=last,
                                    )

                        for kp in kpairs:
                            pss = [spool.tile([P, 2 * NQ], F32, tag="spool",
                                              name=f"ps{_n}")
                                   for _n in range(2)]
                            for j, kb in enumerate(kp):
                                cls = cls_of(qt, kb)
                                qvar, kvar = ((q_lo, k_lo) if cls != "hi"
                                              else (q_hi, k_hi))
                                ksl = slice(kb * P, (kb + 1) * P)
                                jsl = slice(j * NQ, (j + 1) * NQ)
                                for x in range(2):
                                    psl = slice(x * HD, (x + 1) * HD)
                                    nc.tensor.matmul(
                                        pss[x][:, jsl], kvar[psl, pg, ksl],
                                        qvar[psl, pg, qsl],
                                        start=True, stop=True,
                                    )
                            e_aps = [None, None]
                            both_cross = all(cls_of(qt, kb) == "cross"
                                             for kb in kp)
                            for x in range(2):
                                if both_cross:
                                    off = (kp[0] - 4 * qt) * NQ
                                    nc.vector.tensor_tensor(
                                        pss[x][:], pss[x][:],
                                        corr_sb[:, off:off + 2 * NQ],
                                        mybir.AluOpType.mult)
                                elif any(cls_of(qt, kb) == "cross"
                                         for kb in kp):
                                    for j, kb in enumerate(kp):
                                        if cls_of(qt, kb) != "cross":
                                            continue
                                        jsl = slice(j * NQ, (j + 1) * NQ)
                                        off = (kb - 4 * qt) * NQ
                                        nc.vector.tensor_tensor(
                                            pss[x][:, jsl], pss[x][:, jsl],
                                            corr_sb[:, off:off + NQ],
                                            mybir.AluOpType.mult)
                                e_t = epool.tile([P, 2 * NQ], BF16, tag="e")
                                nc.scalar.activation(
                                    e_t[:], pss[x][:],
                                    mybir.ActivationFunctionType.Exp)
                                e_aps[x] = e_t
                            if len(pend) >= 1:
                                emit_vmms(*pend.pop(0))
                            for _d in range(3):
                                nc.tensor.ldweights(q_lo[:, 0, 0:P])
                            pend.append((kp, e_aps))
                        for pe_ in pend:
                            emit_vmms(*pe_)

                        # merge accumulators + far injection + normalize
                        for x in range(2):
                            col = FARCOL[qt]
                            stg = npool.tile([P, NQ], F32, tag="stg")
                            nc.scalar.add(
                                stg[0:HD + 1, :], opo[x][0:HD + 1, :],
                                qsum[0:HD + 1, 2 * pg + x, col:col + 1])
                            nc.vector.tensor_tensor(
                                stg[0:HD + 1, :], ope[x][0:HD + 1, :],
                                stg[0:HD + 1, :], mybir.AluOpType.add)
                            dsum = dram_p.tile([1, NQ], F32, tag="dsum")
                            nc.sync.dma_start(dsum[:], stg[HD:HD + 1, :])
                            srep = npool.tile([HD, NQ], F32, tag="srep")
                            nc.sync.dma_start(
                                srep[:], dsum[:].to_broadcast((HD, NQ)))
                            rrep = npool.tile([HD, NQ], F32, tag="rrep")
                            nc.vector.reciprocal_approx_fast(rrep[:], srep[:])
                            if x == 0:
                                nc.vector.tensor_tensor(
                                    o_all[0:HD, pg, qsl], stg[0:HD, :],
                                    rrep[:], mybir.AluOpType.mult)
                            else:
                                ob = npool.tile([HD, NQ], BF16, tag="ob")
                                nc.vector.tensor_tensor(
                                    ob[:], stg[0:HD, :], rrep[:],
                                    mybir.AluOpType.mult)
                                nc.sync.dma_start(o_all[HD:P, pg, qsl],
                                                  ob[:])

            # ---- Stage C: output projection -------------------------------
            with tc.tile_pool(name="cpool", bufs=2, space="PSUM") as cpool, \
                 tc.tile_pool(name="fpool", bufs=2) as fpool:
                for mt in range(D // P):
                    pc = cpool.tile([P, T], F32, tag="cps")
                    for kt2 in range(2):
                        for nt in range(NT):
                            nc.tensor.matmul(
                                pc[:, nt * NQ:(nt + 1) * NQ],
                                wo_sb[:, kt2, mt * P:(mt + 1) * P],
                                o_all[:, kt2, nt * NQ:(nt + 1) * NQ],
                                start=(kt2 == 0), stop=(kt2 == 1),
                            )
                    fo = fpool.tile([P, T], BF16, tag="fo")
                    if mt % 2 == 0:
                        nc.vector.tensor_copy(fo[:], pc[:])
                    else:
                        nc.scalar.copy(fo[:], pc[:])
                    nc.sync.dma_start(out_ext[mt * P:(mt + 1) * P, :], fo[:])

    nc.finalize()
    return nc


def _get_graph(kp):
    if kp not in _GRAPH_CACHE:
        _GRAPH_CACHE[kp] = _build(kp)
    return _GRAPH_CACHE[kp]


def _install_trace_hooks():
    import types
    import antenv
    if "antenv.axon_hooks" not in sys.modules:
        hooks = types.ModuleType("antenv.axon_hooks")
        hooks._hook = None
        hooks.set_axon_ntff_profile_hook = lambda h: setattr(hooks, "_hook", h)
        hooks.get_axon_ntff_profile_hook = lambda: hooks._hook
        sys.modules["antenv.axon_hooks"] = hooks
        antenv.axon_hooks = hooks
    if sys.modules["antenv.axon_hooks"]._hook is None:
        if "/root/.axon_site" not in sys.path:
            sys.path.insert(0, "/root/.axon_site")
        from trn_agent_boot.trn_boot import _ntff_profile_via_ctypes
        sys.modules["antenv.axon_hooks"].set_axon_ntff_profile_hook(
            _ntff_profile_via_ctypes("/opt/axon/libaxon_pjrt.so"))


def kernel(x, Wq, bq, Wk, bk, Wv, bv, Wo, bo, alpha):
    global LAST_EXEC_NS, LAST_RESULT
    x = np.asarray(x, dtype=np.float32)
    Wq = np.asarray(Wq, dtype=np.float32)
    Wk = np.asarray(Wk, dtype=np.float32)
    Wv = np.asarray(Wv, dtype=np.float32)
    Wo = np.asarray(Wo, dtype=np.float32)
    bq = np.asarray(bq, dtype=np.float32)
    bk = np.asarray(bk, dtype=np.float32)
    bv = np.asarray(bv, dtype=np.float32)
    bo = np.asarray(bo, dtype=np.float32)
    alpha = float(np.asarray(alpha))
    a_eff = alpha * DT_CONST
    scale = HD ** -0.5

    has_bias = bool(np.any(bq) or np.any(bk) or np.any(bv))
    kp = 9 if has_bias else 8
    nc = _get_graph(kp)

    t_idx = np.arange(T, dtype=np.float64)
    e_neg = np.exp(-a_eff * t_idx)
    e_pos = np.exp(+a_eff * t_idx)
    rqlo = np.tile((scale * e_neg).astype(np.float32), (P, 1))
    rqhi = np.tile((scale * e_pos).astype(np.float32), (P, 1))
    rklo = np.tile(e_pos.astype(np.float32), (P, 1))
    rkhi = np.tile(e_neg.astype(np.float32), (P, 1))

    # corr[kk, o*512+qq] = 1 if d>=0 else exp(2*a_eff*d), d = qq-kk-128*o
    kk = np.arange(P)[:, None]
    qq = np.arange(NQ)[None, :]
    corr = np.empty((P, T), dtype=np.float32)
    for o in range(4):
        d = qq - kk - P * o
        corr[:, o * NQ:(o + 1) * NQ] = np.where(
            d >= 0, 1.0, np.exp(2.0 * a_eff * d))

    def wslice(W, b, g):
        ws = W[256 * g:256 * g + 256, :].T.astype(np.float64)
        if has_bias:
            ws = np.vstack([ws, b[256 * g:256 * g + 256][None, :],
                            np.zeros((kp * P - D - 1, 256))])
        return np.ascontiguousarray(ws).astype(ml_dtypes.bfloat16)

    in_maps = []
    for core in range(NCORES):
        b_idx, g = core // 4, core % 4
        xT = x[b_idx].T.astype(np.float64)
        if has_bias:
            xT = np.vstack([xT, np.ones((1, T)), np.zeros((kp * P - D - 1, T))])
        in_maps.append({
            "xT": np.ascontiguousarray(xT).astype(ml_dtypes.bfloat16),
            "wq": wslice(Wq, bq, g),
            "wk": wslice(Wk, bk, g),
            "wv": wslice(Wv, bv, g),
            "wo": np.ascontiguousarray(
                Wo[:, 256 * g:256 * g + 256].T).astype(ml_dtypes.bfloat16),
            "rqlo": rqlo, "rqhi": rqhi, "rklo": rklo, "rkhi": rkhi,
            "corr": corr,
        })

    trace = bool(os.environ.get("BASS_KERNEL_TRACE"))
    if trace:
        _install_trace_hooks()
    res = run_bass_kernel_spmd(nc, in_maps, core_ids=list(range(NCORES)),
                               trace=trace)
    LAST_EXEC_NS = res.exec_time_ns
    LAST_RESULT = res

    out = np.empty((B, T, D), dtype=np.float32)
    for b_idx in range(B):
        acc = np.zeros((D, T), dtype=np.float32)
        for g in range(4):
            acc += np.asarray(res.results[b_idx * 4 + g]["out"],
                              dtype=np.float32)
        out[b_idx] = acc.T + bo[None, :]
    return out


# revision 24
# speedup vs baseline: 1.2071x; 1.0019x over previous
"""AdaptiveFilterAttention on 8 TRN2 NeuronCores.

Sharding: 32 (batch, head) pairs -> 8 cores; core c handles batch c//4,
local head group c%4 (4 heads). Per core: QKV projections for its 256
output dims, per-head attention with exp(-alpha*|i-j|) decay folded in
via rank-1 row scalings of q/k (decay factors exp(+-alpha*t) multiply q
and k rows; diagonal-crossing tiles get a fixup multiply), softmax
without max-subtraction (scores are bounded small), attn@v with an
appended ones-column producing the softmax denominators for free, and a
row-parallel output projection producing a partial (T, D) result summed
on the host across the 4 cores of each batch.
"""
import os
import sys

import numpy as np
import ml_dtypes

sys.path.insert(0, "/opt/trn_rl_repo")

import concourse.bass as bass  # noqa: E402
import concourse.mybir as mybir  # noqa: E402
import concourse.tile as tile  # noqa: E402
from concourse import bacc  # noqa: E402
from concourse.bass_utils import run_bass_kernel_spmd  # noqa: E402

BF16 = mybir.dt.bfloat16
F32 = mybir.dt.float32
P = 128
B, T, D = 2, 2048, 1024
H, HD = 16, 64
HPC = 4            # heads per core
MPC = HD * HPC // P  # 2: partition-tiles of this core's 256 proj dims
NQ = 512           # q free-tile
NT = T // NQ       # 4
KBLK = T // P      # 16 k-blocks of 128
NCORES = 8
DT_CONST = 1.0


LAST_EXEC_NS = None
LAST_RESULT = None
_GRAPH_CACHE = {}


def _build(kp):
    """Build the per-core Bass graph. kp = number of 128-row contraction
    tiles in the projections (8 without bias row, 9 with)."""
    nc = bacc.Bacc(None, target_bir_lowering=False)

    xT_ext = nc.declare_dram_parameter("xT", [kp * P, T], BF16, isOutput=False)
    wq_ext = nc.declare_dram_parameter("wq", [kp * P, 256], BF16, isOutput=False)
    wk_ext = nc.declare_dram_parameter("wk", [kp * P, 256], BF16, isOutput=False)
    wv_ext = nc.declare_dram_parameter("wv", [kp * P, 256], BF16, isOutput=False)
    wo_ext = nc.declare_dram_parameter("wo", [256, D], BF16, isOutput=False)
    rqlo_ext = nc.declare_dram_parameter("rqlo", [P, T], F32, isOutput=False)
    rqhi_ext = nc.declare_dram_parameter("rqhi", [P, T], F32, isOutput=False)
    rklo_ext = nc.declare_dram_parameter("rklo", [P, T], F32, isOutput=False)
    rkhi_ext = nc.declare_dram_parameter("rkhi", [P, T], F32, isOutput=False)
    corr_ext = nc.declare_dram_parameter("corr", [P, T], F32, isOutput=False)
    out_ext = nc.declare_dram_parameter("out", [D, T], BF16, isOutput=True)

    with tile.TileContext(nc) as tc:
        with tc.tile_pool(name="consts", bufs=1) as consts, \
             tc.tile_pool(name="vars", bufs=1) as vars_p, \
             tc.tile_pool(name="dram", bufs=8, space="DRAM") as dram_p:

            xt_sb = consts.tile([P, kp, T], BF16)
            wq_sb = consts.tile([P, kp, 256], BF16)
            wk_sb = consts.tile([P, kp, 256], BF16)
            wv_sb = consts.tile([P, kp, 256], BF16)
            wo_sb = consts.tile([P, 2, D], BF16)
            rqlo = consts.tile([P, T], F32)
            rqhi = consts.tile([P, T], F32)
            rklo = consts.tile([P, T], F32)
            rkhi = consts.tile([P, T], F32)
            corr_sb = consts.tile([P, T], F32)

            for kt in range(kp):
                nc.sync.dma_start(xt_sb[:, kt, :], xT_ext[kt * P:(kt + 1) * P, :])
                nc.sync.dma_start(wq_sb[:, kt, :], wq_ext[kt * P:(kt + 1) * P, :])
                nc.sync.dma_start(wk_sb[:, kt, :], wk_ext[kt * P:(kt + 1) * P, :])
                nc.sync.dma_start(wv_sb[:, kt, :], wv_ext[kt * P:(kt + 1) * P, :])
            for kt2 in range(2):
                nc.sync.dma_start(wo_sb[:, kt2, :], wo_ext[kt2 * P:(kt2 + 1) * P, :])
            nc.sync.dma_start(rqlo[:], rqlo_ext[:])
            nc.sync.dma_start(rqhi[:], rqhi_ext[:])
            nc.sync.dma_start(rklo[:], rklo_ext[:])
            nc.sync.dma_start(rkhi[:], rkhi_ext[:])
            nc.sync.dma_start(corr_sb[:], corr_ext[:])

            # persistent per-core tensors
            q_lo = vars_p.tile([P, MPC, T], BF16)
            q_hi = vars_p.tile([P, MPC, T], BF16)
            k_lo = vars_p.tile([P, MPC, T], BF16)
            k_hi = vars_p.tile([P, MPC, T], BF16)
            v_sb = vars_p.tile([P, KBLK, HPC, HD + 1], BF16)
            o_all = vars_p.tile([P, MPC, T], BF16)

            nc.vector.memset(v_sb[:, :, :, HD:HD + 1], 1.0)
            for _w in range(40):
                nc.tensor.ldweights(q_lo[:, 0, 0:P])

            # ---- Stage A: projections -------------------------------------
            with tc.tile_pool(name="psA", bufs=4, space="PSUM") as psA, \
                 tc.tile_pool(name="psV", bufs=2, space="PSUM") as psV:
                for w_t, lo_r, hi_r, lo_d, hi_d in (
                    (wq_sb, rqlo, rqhi, q_lo, q_hi),
                    (wk_sb, rklo, rkhi, k_lo, k_hi),
                ):
                    for mt in range(MPC):
                        pts = [psA.tile([P, NQ], F32, tag="projps", name=f"pt{_n}")
                               for _n in range(NT)]
                        for kt in range(kp):
                            for nt in range(NT):
                                nc.tensor.matmul(
                                    pts[nt],
                                    w_t[:, kt, mt * P:(mt + 1) * P],
                                    xt_sb[:, kt, nt * NQ:(nt + 1) * NQ],
                                    start=(kt == 0), stop=(kt == kp - 1),
                                )
                        for nt in range(NT):
                            sl = slice(nt * NQ, (nt + 1) * NQ)
                            nc.vector.tensor_tensor(
                                lo_d[:, mt, sl], pts[nt], lo_r[:, sl],
                                mybir.AluOpType.mult)
                            nc.vector.tensor_tensor(
                                hi_d[:, mt, sl], pts[nt], hi_r[:, sl],
                                mybir.AluOpType.mult)
                # v projection: x^T-stationary so v lands [token, dim]
                for mt in range(KBLK):
                    pv = psV.tile([P, 256], F32, tag="vps")
                    for kt in range(kp):
                        nc.tensor.matmul(
                            pv,
                            xt_sb[:, kt, mt * P:(mt + 1) * P],
                            wv_sb[:, kt, :],
                            start=(kt == 0), stop=(kt == kp - 1),
                        )
                    nc.vector.tensor_copy(
                        v_sb[:, mt, :, 0:HD],
                        pv.rearrange("p (h d) -> p h d", h=HPC),
                    )

            # ---- Stage B: attention ---------------------------------------
            # Far-from-diagonal k-blocks: decay ~ 0 so E = exp(~0) ~ 1.
            # Their contribution to O (incl. the softmax denominator via the
            # ones column of v') is a q-independent vector: quarter-aligned
            # prefix sums of per-k-block v' column sums, injected as an ACT
            # bias during the accumulator merge. Near blocks: scores as
            # row-packed K=64 pairs (two heads concurrently), exp on ACT,
            # then attn@v as row-packed K=64 pairs (even/odd token halves
            # into separate PSUM accumulators, merged during normalization).
            FAR_TAU = 512

            def far(qt, kb):
                lo = 512 * qt - 128 * kb - 127   # min Delta when q above k
                hi = 128 * kb - 512 * qt - 511   # min -Delta when k above q
                return lo >= FAR_TAU or hi >= FAR_TAU

            def near_kbs(qt):
                return [kb for kb in range(KBLK) if not far(qt, kb)]

            def cls_of(qt, kb):
                c = kb // 4
                if c < qt:
                    return "lo"
                if c == qt:
                    return "cross"
                return "hi"

            # farsum combos per head: cols 0..3 = quarter sums Q0..Q3 of
            # vcolsum, col 4 = Q0+Q1, col 5 = Q2+Q3.
            # far set: qt0 -> Q2+Q3 (col5), qt1 -> Q3 (col3),
            #          qt2 -> Q0 (col0), qt3 -> Q0+Q1 (col4)
            FARCOL = {0: 5, 1: 3, 2: 0, 3: 4}
            ones_col = vars_p.tile([P, 1], BF16)
            nc.vector.memset(ones_col[:], 1.0)
            qsum = vars_p.tile([P, HPC, 6], F32)
            with tc.tile_pool(name="qpool", bufs=2, space="PSUM") as qpool:
                for h in range(HPC):
                    pf = qpool.tile([P, 4], F32, tag="pf")
                    for g in range(4):
                        for kk in range(4):
                            nc.tensor.matmul(
                                pf[0:HD + 1, g:g + 1],
                                v_sb[:, 4 * g + kk, h, :],
                                ones_col[:],
                                start=(kk == 0), stop=(kk == 3),
                            )
                    nc.vector.tensor_copy(qsum[0:HD + 1, h, 0:4],
                                          pf[0:HD + 1, :])
                    nc.vector.tensor_tensor(
                        qsum[0:HD + 1, h, 4:5], qsum[0:HD + 1, h, 0:1],
                        qsum[0:HD + 1, h, 1:2], mybir.AluOpType.add)
                    nc.vector.tensor_tensor(
                        qsum[0:HD + 1, h, 5:6], qsum[0:HD + 1, h, 2:3],
                        qsum[0:HD + 1, h, 3:4], mybir.AluOpType.add)

            with tc.tile_pool(name="spool", bufs=2, space="PSUM") as spool, \
                 tc.tile_pool(name="opool", bufs=4, space="PSUM") as opool, \
                 tc.tile_pool(name="epool", bufs=4) as epool, \
                 tc.tile_pool(name="npool", bufs=4) as npool:
                for pg in range(MPC):
                    for qt in range(NT):
                        qsl = slice(qt * NQ, (qt + 1) * NQ)
                        nears = near_kbs(qt)
                        kpairs = [(nears[2 * i], nears[2 * i + 1])
                                  for i in range(len(nears) // 2)]
                        ope = [opool.tile([P, NQ], F32, tag="ops",
                                          name=f"ope{_n}") for _n in range(2)]
                        opo = [opool.tile([P, NQ], F32, tag="ops",
                                          name=f"opo{_n}") for _n in range(2)]
                        pend = []

                        def emit_vmms(pkp, paps):
                            first_p = pkp[0] == nears[0]
                            last_p = pkp[1] == nears[-1]
                            for x in range(2):
                                for j, pkb in enumerate(pkp):
                                    jsl = slice(j * NQ, (j + 1) * NQ)
                                    first = first_p and j == 0
                                    last = last_p and j == 1
                                    nc.tensor.matmul(
                                        ope[x][0:HD + 1, :],
                                        v_sb[0:HD, pkb, 2 * pg + x, :],
                                        paps[x][0:HD, jsl],
                                        start=first, stop=last,
                                    )
                                    nc.tensor.matmul(
                                        opo[x][0:HD + 1, :],
                                        v_sb[HD:P, pkb, 2 * pg + x, :],
                                        paps[x][HD:P, jsl],
                                        start=first, stop=last,
                                    )

                        for kp in kpairs:
                            pss = [spool.tile([P, 2 * NQ], F32, tag="spool",
                                              name=f"ps{_n}")
                                   for _n in range(2)]
                            for j, kb in enumerate(kp):
                                cls = cls_of(qt, kb)
                                qvar, kvar = ((q_lo, k_lo) if cls != "hi"
                                              else (q_hi, k_hi))
                                ksl = slice(kb * P, (kb + 1) * P)
                                jsl = slice(j * NQ, (j + 1) * NQ)
                                for x in range(2):
                                    psl = slice(x * HD, (x + 1) * HD)
                                    nc.tensor.matmul(
                                        pss[x][:, jsl], kvar[psl, pg, ksl],
                                        qvar[psl, pg, qsl],
                                        start=True, stop=True,
                                    )
                            e_aps = [None, None]
                            both_cross = all(cls_of(qt, kb) == "cross"
                                             for kb in kp)
                            for x in range(2):
                                if both_cross:
                                    off = (kp[0] - 4 * qt) * NQ
                                    nc.vector.tensor_tensor(
                                        pss[x][:], pss[x][:],
                                        corr_sb[:, off:off + 2 * NQ],
                                        mybir.AluOpType.mult)
                                elif any(cls_of(qt, kb) == "cross"
                                         for kb in kp):
                                    for j, kb in enumerate(kp):
                                        if cls_of(qt, kb) != "cross":
                                            continue
                                        jsl = slice(j * NQ, (j + 1) * NQ)
                                        off = (kb - 4 * qt) * NQ
                                        nc.vector.tensor_tensor(
                                            pss[x][:, jsl], pss[x][:, jsl],
                                            corr_sb[:, off:off + NQ],
                                            mybir.AluOpType.mult)
                                e_t = epool.tile([P, 2 * NQ], BF16, tag="e")
                                nc.scalar.activation(
                                    e_t[:], pss[x][:],
                                    mybir.ActivationFunctionType.Exp)
                                e_aps[x] = e_t
                            if len(pend) >= 1:
                                emit_vmms(*pend.pop(0))
                            for _d in range(2):
                                nc.tensor.ldweights(q_lo[:, 0, 0:P])
                            pend.append((kp, e_aps))
                        for pe_ in pend:
                            emit_vmms(*pe_)

                        # merge accumulators + far injection + normalize
                        for x in range(2):
                            col = FARCOL[qt]
                            stg = npool.tile([P, NQ], F32, tag="stg")
                            nc.scalar.add(
                                stg[0:HD + 1, :], opo[x][0:HD + 1, :],
                                qsum[0:HD + 1, 2 * pg + x, col:col + 1])
                            nc.vector.tensor_tensor(
                                stg[0:HD + 1, :], ope[x][0:HD + 1, :],
                                stg[0:HD + 1, :], mybir.AluOpType.add)
                            dsum = dram_p.tile([1, NQ], F32, tag="dsum")
                            nc.sync.dma_start(dsum[:], stg[HD:HD + 1, :])
                            srep = npool.tile([HD, NQ], F32, tag="srep")
                            nc.sync.dma_start(
                                srep[:], dsum[:].to_broadcast((HD, NQ)))
                            rrep = npool.tile([HD, NQ], F32, tag="rrep")
                            nc.vector.reciprocal_approx_fast(rrep[:], srep[:])
                            if x == 0:
                                nc.vector.tensor_tensor(
                                    o_all[0:HD, pg, qsl], stg[0:HD, :],
                                    rrep[:], mybir.AluOpType.mult)
                            else:
                                ob = npool.tile([HD, NQ], BF16, tag="ob")
                                nc.vector.tensor_tensor(
                                    ob[:], stg[0:HD, :], rrep[:],
                                    mybir.AluOpType.mult)
                                nc.sync.dma_start(o_all[HD:P, pg, qsl],
                                                  ob[:])

            # ---- Stage C: output projection -------------------------------
            with tc.tile_pool(name="cpool", bufs=2, space="PSUM") as cpool, \
                 tc.tile_pool(name="fpool", bufs=2) as fpool:
                for mt in range(D // P):
                    pc = cpool.tile([P, T], F32, tag="cps")
                    for kt2 in range(2):
                        for nt in range(NT):
                            nc.tensor.matmul(
                                pc[:, nt * NQ:(nt + 1) * NQ],
                                wo_sb[:, kt2, mt * P:(mt + 1) * P],
                                o_all[:, kt2, nt * NQ:(nt + 1) * NQ],
                                start=(kt2 == 0), stop=(kt2 == 1),
                            )
                    fo = fpool.tile([P, T], BF16, tag="fo")
                    if mt % 2 == 0:
                        nc.vector.tensor_copy(fo[:], pc[:])
                    else:
                        nc.scalar.copy(fo[:], pc[:])
                    nc.sync.dma_start(out_ext[mt * P:(mt + 1) * P, :], fo[:])

    nc.finalize()
    return nc


def _get_graph(kp):
    if kp not in _GRAPH_CACHE:
        _GRAPH_CACHE[kp] = _build(kp)
    return _GRAPH_CACHE[kp]


def _install_trace_hooks():
    import types
    import antenv
    if "antenv.axon_hooks" not in sys.modules:
        hooks = types.ModuleType("antenv.axon_hooks")
        hooks._hook = None
        hooks.set_axon_ntff_profile_hook = lambda h: setattr(hooks, "_hook", h)
        hooks.get_axon_ntff_profile_hook = lambda: hooks._hook
        sys.modules["antenv.axon_hooks"] = hooks
        antenv.axon_hooks = hooks
    if sys.modules["antenv.axon_hooks"]._hook is None:
        if "/root/.axon_site" not in sys.path:
            sys.path.insert(0, "/root/.axon_site")
        from trn_agent_boot.trn_boot import _ntff_profile_via_ctypes
        sys.modules["antenv.axon_hooks"].set_axon_ntff_profile_hook(
            _ntff_profile_via_ctypes("/opt/axon/libaxon_pjrt.so"))


def kernel(x, Wq, bq, Wk, bk, Wv, bv, Wo, bo, alpha):
    global LAST_EXEC_NS, LAST_RESULT
    x = np.asarray(x, dtype=np.float32)
    Wq = np.asarray(Wq, dtype=np.float32)
    Wk = np.asarray(Wk, dtype=np.float32)
    Wv = np.asarray(Wv, dtype=np.float32)
    Wo = np.asarray(Wo, dtype=np.float32)
    bq = np.asarray(bq, dtype=np.float32)
    bk = np.asarray(bk, dtype=np.float32)
    bv = np.asarray(bv, dtype=np.float32)
    bo = np.asarray(bo, dtype=np.float32)
    alpha = float(np.asarray(alpha))
    a_eff = alpha * DT_CONST
    scale = HD ** -0.5

    has_bias = bool(np.any(bq) or np.any(bk) or np.any(bv))
    kp = 9 if has_bias else 8
    nc = _get_graph(kp)

    t_idx = np.arange(T, dtype=np.float64)
    e_neg = np.exp(-a_eff * t_idx)
    e_pos = np.exp(+a_eff * t_idx)
    rqlo = np.tile((scale * e_neg).astype(np.float32), (P, 1))
    rqhi = np.tile((scale * e_pos).astype(np.float32), (P, 1))
    rklo = np.tile(e_pos.astype(np.float32), (P, 1))
    rkhi = np.tile(e_neg.astype(np.float32), (P, 1))

    # corr[kk, o*512+qq] = 1 if d>=0 else exp(2*a_eff*d), d = qq-kk-128*o
    kk = np.arange(P)[:, None]
    qq = np.arange(NQ)[None, :]
    corr = np.empty((P, T), dtype=np.float32)
    for o in range(4):
        d = qq - kk - P * o
        corr[:, o * NQ:(o + 1) * NQ] = np.where(
            d >= 0, 1.0, np.exp(2.0 * a_eff * d))

    def wslice(W, b, g):
        ws = W[256 * g:256 * g + 256, :].T.astype(np.float64)
        if has_bias:
            ws = np.vstack([ws, b[256 * g:256 * g + 256][None, :],
                            np.zeros((kp * P - D - 1, 256))])
        return np.ascontiguousarray(ws).astype(ml_dtypes.bfloat16)

    in_maps = []
    for core in range(NCORES):
        b_idx, g = core // 4, core % 4
        xT = x[b_idx].T.astype(np.float64)
        if has_bias:
            xT = np.vstack([xT, np.ones((1, T)), np.zeros((kp * P - D - 1, T))])
        in_maps.append({
            "xT": np.ascontiguousarray(xT).astype(ml_dtypes.bfloat16),
            "wq": wslice(Wq, bq, g),
            "wk": wslice(Wk, bk, g),
            "wv": wslice(Wv, bv, g),
            "wo": np.ascontiguousarray(
                Wo[:, 256 * g:256 * g + 256].T).astype(ml_dtypes.bfloat16),
            "rqlo": rqlo, "rqhi": rqhi, "rklo": rklo, "rkhi": rkhi,
            "corr": corr,
        })

    trace = bool(os.environ.get("BASS_KERNEL_TRACE"))
    if trace:
        _install_trace_hooks()
    res = run_bass_kernel_spmd(nc, in_maps, core_ids=list(range(NCORES)),
                               trace=trace)
    LAST_EXEC_NS = res.exec_time_ns
    LAST_RESULT = res

    out = np.empty((B, T, D), dtype=np.float32)
    for b_idx in range(B):
        acc = np.zeros((D, T), dtype=np.float32)
        for g in range(4):
            acc += np.asarray(res.results[b_idx * 4 + g]["out"],
                              dtype=np.float32)
        out[b_idx] = acc.T + bo[None, :]
    return out


# revision 34
# speedup vs baseline: 1.3650x; 1.1309x over previous
"""AdaptiveFilterAttention on 8 TRN2 NeuronCores.

Sharding: 32 (batch, head) pairs -> 8 cores; core c handles batch c//4,
local head group c%4 (4 heads). Per core: QKV projections for its 256
output dims, per-head attention with exp(-alpha*|i-j|) decay folded in
via rank-1 row scalings of q/k (decay factors exp(+-alpha*t) multiply q
and k rows; diagonal-crossing tiles get a fixup multiply), softmax
without max-subtraction (scores are bounded small), attn@v with an
appended ones-column producing the softmax denominators for free, and a
row-parallel output projection producing a partial (T, D) result summed
on the host across the 4 cores of each batch.
"""
import os
import sys

import numpy as np
import ml_dtypes

sys.path.insert(0, "/opt/trn_rl_repo")

import concourse.bass as bass  # noqa: E402
import concourse.mybir as mybir  # noqa: E402
import concourse.tile as tile  # noqa: E402
from concourse import bacc  # noqa: E402
from concourse.bass_utils import run_bass_kernel_spmd  # noqa: E402

BF16 = mybir.dt.bfloat16
F32 = mybir.dt.float32
P = 128
B, T, D = 2, 2048, 1024
H, HD = 16, 64
HPC = 4            # heads per core
MPC = HD * HPC // P  # 2: partition-tiles of this core's 256 proj dims
NQ = 512           # q free-tile
NT = T // NQ       # 4
KBLK = T // P      # 16 k-blocks of 128
NCORES = 8
DT_CONST = 1.0


LAST_EXEC_NS = None
LAST_RESULT = None
_GRAPH_CACHE = {}


def _build(kp):
    """Build the per-core Bass graph. kp = number of 128-row contraction
    tiles in the projections (8 without bias row, 9 with)."""
    nc = bacc.Bacc(None, target_bir_lowering=False)

    xT_ext = nc.declare_dram_parameter("xT", [kp * P, T], BF16, isOutput=False)
    wq_ext = nc.declare_dram_parameter("wq", [kp * P, 256], BF16, isOutput=False)
    wk_ext = nc.declare_dram_parameter("wk", [kp * P, 256], BF16, isOutput=False)
    wv_ext = nc.declare_dram_parameter("wv", [kp * P, 256], BF16, isOutput=False)
    wo_ext = nc.declare_dram_parameter("wo", [256, D], BF16, isOutput=False)
    rqlo_ext = nc.declare_dram_parameter("rqlo", [P, T], F32, isOutput=False)
    rqhi_ext = nc.declare_dram_parameter("rqhi", [P, T], F32, isOutput=False)
    rklo_ext = nc.declare_dram_parameter("rklo", [P, T], F32, isOutput=False)
    rkhi_ext = nc.declare_dram_parameter("rkhi", [P, T], F32, isOutput=False)
    corr_ext = nc.declare_dram_parameter("corr", [P, T], F32, isOutput=False)
    out_ext = nc.declare_dram_parameter("out", [D, T], BF16, isOutput=True)

    with tile.TileContext(nc) as tc:
        with tc.tile_pool(name="consts", bufs=1) as consts, \
             tc.tile_pool(name="vars", bufs=1) as vars_p, \
             tc.tile_pool(name="dram", bufs=8, space="DRAM") as dram_p:

            xt_sb = consts.tile([P, kp, T], BF16)
            wq_sb = consts.tile([P, kp, 256], BF16)
            wk_sb = consts.tile([P, kp, 256], BF16)
            wv_sb = consts.tile([P, kp, 256], BF16)
            wo_sb = consts.tile([P, 2, D], BF16)
            rqlo = consts.tile([P, T], F32)
            rqhi = consts.tile([P, T], F32)
            rklo = consts.tile([P, T], F32)
            rkhi = consts.tile([P, T], F32)
            corr_sb = consts.tile([P, T], F32)

            for kt in range(kp):
                nc.sync.dma_start(xt_sb[:, kt, :], xT_ext[kt * P:(kt + 1) * P, :])
                nc.sync.dma_start(wq_sb[:, kt, :], wq_ext[kt * P:(kt + 1) * P, :])
                nc.sync.dma_start(wk_sb[:, kt, :], wk_ext[kt * P:(kt + 1) * P, :])
                nc.sync.dma_start(wv_sb[:, kt, :], wv_ext[kt * P:(kt + 1) * P, :])
            for kt2 in range(2):
                nc.sync.dma_start(wo_sb[:, kt2, :], wo_ext[kt2 * P:(kt2 + 1) * P, :])
            nc.sync.dma_start(rqlo[:], rqlo_ext[:])
            nc.sync.dma_start(rqhi[:], rqhi_ext[:])
            nc.sync.dma_start(rklo[:], rklo_ext[:])
            nc.sync.dma_start(rkhi[:], rkhi_ext[:])
            nc.sync.dma_start(corr_sb[:], corr_ext[:])

            # persistent per-core tensors
            q_lo = vars_p.tile([P, MPC, T], BF16)
            q_hi = vars_p.tile([P, MPC, T], BF16)
            k_lo = vars_p.tile([P, MPC, T], BF16)
            k_hi = vars_p.tile([P, MPC, T], BF16)
            v_sb = vars_p.tile([P, KBLK, HPC, HD + 1], BF16)
            o_all = vars_p.tile([P, MPC, T], BF16)

            nc.vector.memset(v_sb[:, :, :, HD:HD + 1], 1.0)
            for _w in range(40):
                nc.tensor.ldweights(q_lo[:, 0, 0:P])

            # ---- Stage A: projections -------------------------------------
            with tc.tile_pool(name="psA", bufs=4, space="PSUM") as psA, \
                 tc.tile_pool(name="psV", bufs=2, space="PSUM") as psV:
                for w_t, lo_r, hi_r, lo_d, hi_d in (
                    (wq_sb, rqlo, rqhi, q_lo, q_hi),
                    (wk_sb, rklo, rkhi, k_lo, k_hi),
                ):
                    for mt in range(MPC):
                        pts = [psA.tile([P, NQ], F32, tag="projps", name=f"pt{_n}")
                               for _n in range(NT)]
                        for kt in range(kp):
                            for nt in range(NT):
                                nc.tensor.matmul(
                                    pts[nt],
                                    w_t[:, kt, mt * P:(mt + 1) * P],
                                    xt_sb[:, kt, nt * NQ:(nt + 1) * NQ],
                                    start=(kt == 0), stop=(kt == kp - 1),
                                )
                        for nt in range(NT):
                            sl = slice(nt * NQ, (nt + 1) * NQ)
                            nc.vector.tensor_tensor(
                                lo_d[:, mt, sl], pts[nt], lo_r[:, sl],
                                mybir.AluOpType.mult)
                            nc.vector.tensor_tensor(
                                hi_d[:, mt, sl], pts[nt], hi_r[:, sl],
                                mybir.AluOpType.mult)
                # v projection: x^T-stationary so v lands [token, dim]
                for mt in range(KBLK):
                    pv = psV.tile([P, 256], F32, tag="vps")
                    for kt in range(kp):
                        nc.tensor.matmul(
                            pv,
                            xt_sb[:, kt, mt * P:(mt + 1) * P],
                            wv_sb[:, kt, :],
                            start=(kt == 0), stop=(kt == kp - 1),
                        )
                    nc.vector.tensor_copy(
                        v_sb[:, mt, :, 0:HD],
                        pv.rearrange("p (h d) -> p h d", h=HPC),
                    )

            # ---- Stage B: attention ---------------------------------------
            # Far-from-diagonal k-blocks: decay ~ 0 so E = exp(~0) ~ 1.
            # Their contribution to O (incl. the softmax denominator via the
            # ones column of v') is a q-independent vector: quarter-aligned
            # prefix sums of per-k-block v' column sums, injected as an ACT
            # bias during the accumulator merge. Near blocks: scores as
            # row-packed K=64 pairs (two heads concurrently), exp on ACT,
            # then attn@v as row-packed K=64 pairs (even/odd token halves
            # into separate PSUM accumulators, merged during normalization).
            FAR_TAU = 512

            def far(qt, kb):
                lo = 512 * qt - 128 * kb - 127   # min Delta when q above k
                hi = 128 * kb - 512 * qt - 511   # min -Delta when k above q
                return lo >= FAR_TAU or hi >= FAR_TAU

            def near_kbs(qt):
                return [kb for kb in range(KBLK) if not far(qt, kb)]

            def cls_of(qt, kb):
                c = kb // 4
                if c < qt:
                    return "lo"
                if c == qt:
                    return "cross"
                return "hi"

            # farsum combos per head: cols 0..3 = quarter sums Q0..Q3 of
            # vcolsum, col 4 = Q0+Q1, col 5 = Q2+Q3.
            # far set: qt0 -> Q2+Q3 (col5), qt1 -> Q3 (col3),
            #          qt2 -> Q0 (col0), qt3 -> Q0+Q1 (col4)
            FARCOL = {0: 5, 1: 3, 2: 0, 3: 4}
            ones_col = vars_p.tile([P, 1], BF16)
            nc.vector.memset(ones_col[:], 1.0)
            qsum = vars_p.tile([P, HPC, 6], F32)
            with tc.tile_pool(name="qpool", bufs=2, space="PSUM") as qpool:
                for h in range(HPC):
                    pf = qpool.tile([P, 4], F32, tag="pf")
                    for g in range(4):
                        for kk in range(4):
                            nc.tensor.matmul(
                                pf[0:HD + 1, g:g + 1],
                                v_sb[:, 4 * g + kk, h, :],
                                ones_col[:],
                                start=(kk == 0), stop=(kk == 3),
                            )
                    nc.vector.tensor_copy(qsum[0:HD + 1, h, 0:4],
                                          pf[0:HD + 1, :])
                    nc.vector.tensor_tensor(
                        qsum[0:HD + 1, h, 4:5], qsum[0:HD + 1, h, 0:1],
                        qsum[0:HD + 1, h, 1:2], mybir.AluOpType.add)
                    nc.vector.tensor_tensor(
                        qsum[0:HD + 1, h, 5:6], qsum[0:HD + 1, h, 2:3],
                        qsum[0:HD + 1, h, 3:4], mybir.AluOpType.add)

            with tc.tile_pool(name="spool", bufs=2, space="PSUM") as spool, \
                 tc.tile_pool(name="opool", bufs=4, space="PSUM") as opool, \
                 tc.tile_pool(name="epool", bufs=8) as epool, \
                 tc.tile_pool(name="npool", bufs=4) as npool:

                def emit_norm(pg, qt, accs):
                    qsl = slice(qt * NQ, (qt + 1) * NQ)
                    ope, opo = accs
                    for x in range(2):
                        col = FARCOL[qt]
                        stg = npool.tile([P, NQ], F32, tag="stg")
                        nc.scalar.add(
                            stg[0:HD + 1, :], opo[x][0:HD + 1, :],
                            qsum[0:HD + 1, 2 * pg + x, col:col + 1])
                        nc.vector.tensor_tensor(
                            stg[0:HD + 1, :], ope[x][0:HD + 1, :],
                            stg[0:HD + 1, :], mybir.AluOpType.add)
                        dsum = dram_p.tile([1, NQ], F32, tag="dsum")
                        nc.sync.dma_start(dsum[:], stg[HD:HD + 1, :])
                        srep = npool.tile([HD, NQ], F32, tag="srep")
                        nc.sync.dma_start(
                            srep[:], dsum[:].to_broadcast((HD, NQ)))
                        rrep = npool.tile([HD, NQ], F32, tag="rrep")
                        nc.vector.reciprocal_approx_fast(rrep[:], srep[:])
                        if x == 0:
                            nc.vector.tensor_tensor(
                                o_all[0:HD, pg, qsl], stg[0:HD, :],
                                rrep[:], mybir.AluOpType.mult)
                        else:
                            ob = npool.tile([HD, NQ], BF16, tag="ob")
                            nc.vector.tensor_tensor(
                                ob[:], stg[0:HD, :], rrep[:],
                                mybir.AluOpType.mult)
                            nc.sync.dma_start(o_all[HD:P, pg, qsl], ob[:])

                def emit_vmms(u):
                    pg, qt, kp, accs, e_aps = u
                    nears = near_kbs(qt)
                    ope, opo = accs
                    first_p = kp[0] == nears[0]
                    last_p = kp[1] == nears[-1]
                    for x in range(2):
                        for j, pkb in enumerate(kp):
                            jsl = slice(j * NQ, (j + 1) * NQ)
                            first = first_p and j == 0
                            last = last_p and j == 1
                            nc.tensor.matmul(
                                ope[x][0:HD + 1, :],
                                v_sb[0:HD, pkb, 2 * pg + x, :],
                                e_aps[x][0:HD, jsl],
                                start=first, stop=last,
                            )
                            nc.tensor.matmul(
                                opo[x][0:HD + 1, :],
                                v_sb[HD:P, pkb, 2 * pg + x, :],
                                e_aps[x][HD:P, jsl],
                                start=first, stop=last,
                            )
                    if last_p:
                        emit_norm(pg, qt, accs)

                units = []
                for pg in range(MPC):
                    for qt in range(NT):
                        nears = near_kbs(qt)
                        units.append((pg, qt,
                                      [(nears[2 * i], nears[2 * i + 1])
                                       for i in range(len(nears) // 2)]))

                pend = []
                accs = None
                for pg, qt, kpairs in units:
                    qsl = slice(qt * NQ, (qt + 1) * NQ)
                    accs = ([opool.tile([P, NQ], F32, tag="ops",
                                        name=f"ope{_n}") for _n in range(2)],
                            [opool.tile([P, NQ], F32, tag="ops",
                                        name=f"opo{_n}") for _n in range(2)])
                    for kp in kpairs:
                        pss = [spool.tile([P, 2 * NQ], F32, tag="spool",
                                          name=f"ps{_n}")
                               for _n in range(2)]
                        for j, kb in enumerate(kp):
                            cls = cls_of(qt, kb)
                            qvar, kvar = ((q_lo, k_lo) if cls != "hi"
                                          else (q_hi, k_hi))
                            ksl = slice(kb * P, (kb + 1) * P)
                            jsl = slice(j * NQ, (j + 1) * NQ)
                            for x in range(2):
                                psl = slice(x * HD, (x + 1) * HD)
                                nc.tensor.matmul(
                                    pss[x][:, jsl], kvar[psl, pg, ksl],
                                    qvar[psl, pg, qsl],
                                    start=True, stop=True,
                                )
                        e_aps = [None, None]
                        both_cross = all(cls_of(qt, kb) == "cross"
                                         for kb in kp)
                        for x in range(2):
                            if both_cross:
                                off = (kp[0] - 4 * qt) * NQ
                                nc.vector.tensor_tensor(
                                    pss[x][:], pss[x][:],
                                    corr_sb[:, off:off + 2 * NQ],
                                    mybir.AluOpType.mult)
                            elif any(cls_of(qt, kb) == "cross"
                                     for kb in kp):
                                for j, kb in enumerate(kp):
                                    if cls_of(qt, kb) != "cross":
                                        continue
                                    jsl = slice(j * NQ, (j + 1) * NQ)
                                    off = (kb - 4 * qt) * NQ
                                    nc.vector.tensor_tensor(
                                        pss[x][:, jsl], pss[x][:, jsl],
                                        corr_sb[:, off:off + NQ],
                                        mybir.AluOpType.mult)
                            e_t = epool.tile([P, 2 * NQ], BF16, tag="e")
                            nc.scalar.activation(
                                e_t[:], pss[x][:],
                                mybir.ActivationFunctionType.Exp)
                            e_aps[x] = e_t
                        if len(pend) >= 3:
                            emit_vmms(pend.pop(0))
                        for _d in range(2):
                            nc.tensor.ldweights(q_lo[:, 0, 0:P])
                        pend.append((pg, qt, kp, accs, e_aps))
                for u in pend:
                    emit_vmms(u)

            # ---- Stage C: output projection -------------------------------
            with tc.tile_pool(name="cpool", bufs=2, space="PSUM") as cpool, \
                 tc.tile_pool(name="fpool", bufs=2) as fpool:
                for mt in range(D // P):
                    pc = cpool.tile([P, T], F32, tag="cps")
                    for kt2 in range(2):
                        for nt in range(NT):
                            nc.tensor.matmul(
                                pc[:, nt * NQ:(nt + 1) * NQ],
                                wo_sb[:, kt2, mt * P:(mt + 1) * P],
                                o_all[:, kt2, nt * NQ:(nt + 1) * NQ],
                                start=(kt2 == 0), stop=(kt2 == 1),
                            )
                    fo = fpool.tile([P, T], BF16, tag="fo")
                    if mt % 2 == 0:
                        nc.vector.tensor_copy(fo[:], pc[:])
                    else:
                        nc.scalar.copy(fo[:], pc[:])
                    nc.sync.dma_start(out_ext[mt * P:(mt + 1) * P, :], fo[:])

    nc.finalize()
    return nc


def _get_graph(kp):
    if kp not in _GRAPH_CACHE:
        _GRAPH_CACHE[kp] = _build(kp)
    return _GRAPH_CACHE[kp]


def _install_trace_hooks():
    import types
    import antenv
    if "antenv.axon_hooks" not in sys.modules:
        hooks = types.ModuleType("antenv.axon_hooks")
        hooks._hook = None
        hooks.set_axon_ntff_profile_hook = lambda h: setattr(hooks, "_hook", h)
        hooks.get_axon_ntff_profile_hook = lambda: hooks._hook
        sys.modules["antenv.axon_hooks"] = hooks
        antenv.axon_hooks = hooks
    if sys.modules["antenv.axon_hooks"]._hook is None:
        if "/root/.axon_site" not in sys.path:
            sys.path.insert(0, "/root/.axon_site")
        from trn_agent_boot.trn_boot import _ntff_profile_via_ctypes
        sys.modules["antenv.axon_hooks"].set_axon_ntff_profile_hook(
            _ntff_profile_via_ctypes("/opt/axon/libaxon_pjrt.so"))


def kernel(x, Wq, bq, Wk, bk, Wv, bv, Wo, bo, alpha):
    global LAST_EXEC_NS, LAST_RESULT
    x = np.asarray(x, dtype=np.float32)
    Wq = np.asarray(Wq, dtype=np.float32)
    Wk = np.asarray(Wk, dtype=np.float32)
    Wv = np.asarray(Wv, dtype=np.float32)
    Wo = np.asarray(Wo, dtype=np.float32)
    bq = np.asarray(bq, dtype=np.float32)
    bk = np.asarray(bk, dtype=np.float32)
    bv = np.asarray(bv, dtype=np.float32)
    bo = np.asarray(bo, dtype=np.float32)
    alpha = float(np.asarray(alpha))
    a_eff = alpha * DT_CONST
    scale = HD ** -0.5

    has_bias = bool(np.any(bq) or np.any(bk) or np.any(bv))
    kp = 9 if has_bias else 8
    nc = _get_graph(kp)

    t_idx = np.arange(T, dtype=np.float64)
    e_neg = np.exp(-a_eff * t_idx)
    e_pos = np.exp(+a_eff * t_idx)
    rqlo = np.tile((scale * e_neg).astype(np.float32), (P, 1))
    rqhi = np.tile((scale * e_pos).astype(np.float32), (P, 1))
    rklo = np.tile(e_pos.astype(np.float32), (P, 1))
    rkhi = np.tile(e_neg.astype(np.float32), (P, 1))

    # corr[kk, o*512+qq] = 1 if d>=0 else exp(2*a_eff*d), d = qq-kk-128*o
    kk = np.arange(P)[:, None]
    qq = np.arange(NQ)[None, :]
    corr = np.empty((P, T), dtype=np.float32)
    for o in range(4):
        d = qq - kk - P * o
        corr[:, o * NQ:(o + 1) * NQ] = np.where(
            d >= 0, 1.0, np.exp(2.0 * a_eff * d))

    def wslice(W, b, g):
        ws = W[256 * g:256 * g + 256, :].T.astype(np.float64)
        if has_bias:
            ws = np.vstack([ws, b[256 * g:256 * g + 256][None, :],
                            np.zeros((kp * P - D - 1, 256))])
        return np.ascontiguousarray(ws).astype(ml_dtypes.bfloat16)

    in_maps = []
    for core in range(NCORES):
        b_idx, g = core // 4, core % 4
        xT = x[b_idx].T.astype(np.float64)
        if has_bias:
            xT = np.vstack([xT, np.ones((1, T)), np.zeros((kp * P - D - 1, T))])
        in_maps.append({
            "xT": np.ascontiguousarray(xT).astype(ml_dtypes.bfloat16),
            "wq": wslice(Wq, bq, g),
            "wk": wslice(Wk, bk, g),
            "wv": wslice(Wv, bv, g),
            "wo": np.ascontiguousarray(
                Wo[:, 256 * g:256 * g + 256].T).astype(ml_dtypes.bfloat16),
            "rqlo": rqlo, "rqhi": rqhi, "rklo": rklo, "rkhi": rkhi,
            "corr": corr,
        })

    trace = bool(os.environ.get("BASS_KERNEL_TRACE"))
    if trace:
        _install_trace_hooks()
    res = run_bass_kernel_spmd(nc, in_maps, core_ids=list(range(NCORES)),
                               trace=trace)
    LAST_EXEC_NS = res.exec_time_ns
    LAST_RESULT = res

    out = np.empty((B, T, D), dtype=np.float32)
    for b_idx in range(B):
        acc = np.zeros((D, T), dtype=np.float32)
        for g in range(4):
            acc += np.asarray(res.results[b_idx * 4 + g]["out"],
                              dtype=np.float32)
        out[b_idx] = acc.T + bo[None, :]
    return out


# revision 35
# speedup vs baseline: 1.3840x; 1.0139x over previous
"""AdaptiveFilterAttention on 8 TRN2 NeuronCores.

Sharding: 32 (batch, head) pairs -> 8 cores; core c handles batch c//4,
local head group c%4 (4 heads). Per core: QKV projections for its 256
output dims, per-head attention with exp(-alpha*|i-j|) decay folded in
via rank-1 row scalings of q/k (decay factors exp(+-alpha*t) multiply q
and k rows; diagonal-crossing tiles get a fixup multiply), softmax
without max-subtraction (scores are bounded small), attn@v with an
appended ones-column producing the softmax denominators for free, and a
row-parallel output projection producing a partial (T, D) result summed
on the host across the 4 cores of each batch.
"""
import os
import sys

import numpy as np
import ml_dtypes

sys.path.insert(0, "/opt/trn_rl_repo")

import concourse.bass as bass  # noqa: E402
import concourse.mybir as mybir  # noqa: E402
import concourse.tile as tile  # noqa: E402
from concourse import bacc  # noqa: E402
from concourse.bass_utils import run_bass_kernel_spmd  # noqa: E402

BF16 = mybir.dt.bfloat16
F32 = mybir.dt.float32
P = 128
B, T, D = 2, 2048, 1024
H, HD = 16, 64
HPC = 4            # heads per core
MPC = HD * HPC // P  # 2: partition-tiles of this core's 256 proj dims
NQ = 512           # q free-tile
NT = T // NQ       # 4
KBLK = T // P      # 16 k-blocks of 128
NCORES = 8
DT_CONST = 1.0


LAST_EXEC_NS = None
LAST_RESULT = None
_GRAPH_CACHE = {}


def _build(kp):
    """Build the per-core Bass graph. kp = number of 128-row contraction
    tiles in the projections (8 without bias row, 9 with)."""
    nc = bacc.Bacc(None, target_bir_lowering=False)

    xT_ext = nc.declare_dram_parameter("xT", [kp * P, T], BF16, isOutput=False)
    wq_ext = nc.declare_dram_parameter("wq", [kp * P, 256], BF16, isOutput=False)
    wk_ext = nc.declare_dram_parameter("wk", [kp * P, 256], BF16, isOutput=False)
    wv_ext = nc.declare_dram_parameter("wv", [kp * P, 256], BF16, isOutput=False)
    wo_ext = nc.declare_dram_parameter("wo", [256, D], BF16, isOutput=False)
    rqlo_ext = nc.declare_dram_parameter("rqlo", [P, T], F32, isOutput=False)
    rqhi_ext = nc.declare_dram_parameter("rqhi", [P, T], F32, isOutput=False)
    rklo_ext = nc.declare_dram_parameter("rklo", [P, T], F32, isOutput=False)
    rkhi_ext = nc.declare_dram_parameter("rkhi", [P, T], F32, isOutput=False)
    corr_ext = nc.declare_dram_parameter("corr", [P, T], F32, isOutput=False)
    out_ext = nc.declare_dram_parameter("out", [D, T], BF16, isOutput=True)

    with tile.TileContext(nc) as tc:
        with tc.tile_pool(name="consts", bufs=1) as consts, \
             tc.tile_pool(name="vars", bufs=1) as vars_p, \
             tc.tile_pool(name="dram", bufs=8, space="DRAM") as dram_p:

            xt_sb = consts.tile([P, kp, T], BF16)
            wq_sb = consts.tile([P, kp, 256], BF16)
            wk_sb = consts.tile([P, kp, 256], BF16)
            wv_sb = consts.tile([P, kp, 256], BF16)
            wo_sb = consts.tile([P, 2, D], BF16)
            rqlo = consts.tile([P, T], F32)
            rqhi = consts.tile([P, T], F32)
            rklo = consts.tile([P, T], F32)
            rkhi = consts.tile([P, T], F32)
            corr_sb = consts.tile([P, T], F32)

            for kt in range(kp):
                nc.sync.dma_start(xt_sb[:, kt, :], xT_ext[kt * P:(kt + 1) * P, :])
                nc.sync.dma_start(wq_sb[:, kt, :], wq_ext[kt * P:(kt + 1) * P, :])
                nc.sync.dma_start(wk_sb[:, kt, :], wk_ext[kt * P:(kt + 1) * P, :])
                nc.sync.dma_start(wv_sb[:, kt, :], wv_ext[kt * P:(kt + 1) * P, :])
            for kt2 in range(2):
                nc.sync.dma_start(wo_sb[:, kt2, :], wo_ext[kt2 * P:(kt2 + 1) * P, :])
            nc.sync.dma_start(rqlo[:], rqlo_ext[:])
            nc.sync.dma_start(rqhi[:], rqhi_ext[:])
            nc.sync.dma_start(rklo[:], rklo_ext[:])
            nc.sync.dma_start(rkhi[:], rkhi_ext[:])
            nc.sync.dma_start(corr_sb[:], corr_ext[:])

            # persistent per-core tensors
            q_lo = vars_p.tile([P, MPC, T], BF16)
            q_hi = vars_p.tile([P, MPC, T], BF16)
            k_lo = vars_p.tile([P, MPC, T], BF16)
            k_hi = vars_p.tile([P, MPC, T], BF16)
            v_sb = vars_p.tile([P, KBLK, HPC, HD + 1], BF16)
            o_all = vars_p.tile([P, MPC, T], BF16)

            nc.vector.memset(v_sb[:, :, :, HD:HD + 1], 1.0)
            for _w in range(40):
                nc.tensor.ldweights(q_lo[:, 0, 0:P])

            # ---- Stage A: projections -------------------------------------
            with tc.tile_pool(name="psA", bufs=4, space="PSUM") as psA, \
                 tc.tile_pool(name="psV", bufs=2, space="PSUM") as psV:
                for w_t, lo_r, hi_r, lo_d, hi_d in (
                    (wq_sb, rqlo, rqhi, q_lo, q_hi),
                    (wk_sb, rklo, rkhi, k_lo, k_hi),
                ):
                    for mt in range(MPC):
                        pts = [psA.tile([P, NQ], F32, tag="projps", name=f"pt{_n}")
                               for _n in range(NT)]
                        for kt in range(kp):
                            for nt in range(NT):
                                nc.tensor.matmul(
                                    pts[nt],
                                    w_t[:, kt, mt * P:(mt + 1) * P],
                                    xt_sb[:, kt, nt * NQ:(nt + 1) * NQ],
                                    start=(kt == 0), stop=(kt == kp - 1),
                                )
                        for nt in range(NT):
                            sl = slice(nt * NQ, (nt + 1) * NQ)
                            nc.vector.tensor_tensor(
                                lo_d[:, mt, sl], pts[nt], lo_r[:, sl],
                                mybir.AluOpType.mult)
                            nc.vector.tensor_tensor(
                                hi_d[:, mt, sl], pts[nt], hi_r[:, sl],
                                mybir.AluOpType.mult)
                # v projection: x^T-stationary so v lands [token, dim]
                for mt in range(KBLK):
                    pv = psV.tile([P, 256], F32, tag="vps")
                    for kt in range(kp):
                        nc.tensor.matmul(
                            pv,
                            xt_sb[:, kt, mt * P:(mt + 1) * P],
                            wv_sb[:, kt, :],
                            start=(kt == 0), stop=(kt == kp - 1),
                        )
                    nc.vector.tensor_copy(
                        v_sb[:, mt, :, 0:HD],
                        pv.rearrange("p (h d) -> p h d", h=HPC),
                    )

            # ---- Stage B: attention ---------------------------------------
            # Far-from-diagonal k-blocks: decay ~ 0 so E = exp(~0) ~ 1.
            # Their contribution to O (incl. the softmax denominator via the
            # ones column of v') is a q-independent vector: quarter-aligned
            # prefix sums of per-k-block v' column sums, injected as an ACT
            # bias during the accumulator merge. Near blocks: scores as
            # row-packed K=64 pairs (two heads concurrently), exp on ACT,
            # then attn@v as row-packed K=64 pairs (even/odd token halves
            # into separate PSUM accumulators, merged during normalization).
            FAR_TAU = 512

            def far(qt, kb):
                lo = 512 * qt - 128 * kb - 127   # min Delta when q above k
                hi = 128 * kb - 512 * qt - 511   # min -Delta when k above q
                return lo >= FAR_TAU or hi >= FAR_TAU

            def near_kbs(qt):
                return [kb for kb in range(KBLK) if not far(qt, kb)]

            def cls_of(qt, kb):
                c = kb // 4
                if c < qt:
                    return "lo"
                if c == qt:
                    return "cross"
                return "hi"

            # farsum combos per head: cols 0..3 = quarter sums Q0..Q3 of
            # vcolsum, col 4 = Q0+Q1, col 5 = Q2+Q3.
            # far set: qt0 -> Q2+Q3 (col5), qt1 -> Q3 (col3),
            #          qt2 -> Q0 (col0), qt3 -> Q0+Q1 (col4)
            FARCOL = {0: 5, 1: 3, 2: 0, 3: 4}
            ones_col = vars_p.tile([P, 1], BF16)
            nc.vector.memset(ones_col[:], 1.0)
            qsum = vars_p.tile([P, HPC, 6], F32)
            with tc.tile_pool(name="qpool", bufs=2, space="PSUM") as qpool:
                for h in range(HPC):
                    pf = qpool.tile([P, 4], F32, tag="pf")
                    for g in range(4):
                        for kk in range(4):
                            nc.tensor.matmul(
                                pf[0:HD + 1, g:g + 1],
                                v_sb[:, 4 * g + kk, h, :],
                                ones_col[:],
                                start=(kk == 0), stop=(kk == 3),
                            )
                    nc.vector.tensor_copy(qsum[0:HD + 1, h, 0:4],
                                          pf[0:HD + 1, :])
                    nc.vector.tensor_tensor(
                        qsum[0:HD + 1, h, 4:5], qsum[0:HD + 1, h, 0:1],
                        qsum[0:HD + 1, h, 1:2], mybir.AluOpType.add)
                    nc.vector.tensor_tensor(
                        qsum[0:HD + 1, h, 5:6], qsum[0:HD + 1, h, 2:3],
                        qsum[0:HD + 1, h, 3:4], mybir.AluOpType.add)

            with tc.tile_pool(name="spool", bufs=2, space="PSUM") as spool, \
                 tc.tile_pool(name="opool", bufs=4, space="PSUM") as opool, \
                 tc.tile_pool(name="epool", bufs=8) as epool, \
                 tc.tile_pool(name="npool", bufs=4) as npool:

                def emit_norm(pg, qt, accs):
                    qsl = slice(qt * NQ, (qt + 1) * NQ)
                    ope, opo = accs
                    for x in range(2):
                        col = FARCOL[qt]
                        stg = npool.tile([P, NQ], F32, tag="stg")
                        nc.scalar.add(
                            stg[0:HD + 1, :], opo[x][0:HD + 1, :],
                            qsum[0:HD + 1, 2 * pg + x, col:col + 1])
                        nc.vector.tensor_tensor(
                            stg[0:HD + 1, :], ope[x][0:HD + 1, :],
                            stg[0:HD + 1, :], mybir.AluOpType.add)
                        dsum = dram_p.tile([1, NQ], F32, tag="dsum")
                        nc.sync.dma_start(dsum[:], stg[HD:HD + 1, :])
                        srep = npool.tile([HD, NQ], F32, tag="srep")
                        nc.sync.dma_start(
                            srep[:], dsum[:].to_broadcast((HD, NQ)))
                        rrep = npool.tile([HD, NQ], F32, tag="rrep")
                        nc.vector.reciprocal_approx_fast(rrep[:], srep[:])
                        if x == 0:
                            nc.vector.tensor_tensor(
                                o_all[0:HD, pg, qsl], stg[0:HD, :],
                                rrep[:], mybir.AluOpType.mult)
                        else:
                            ob = npool.tile([HD, NQ], BF16, tag="ob")
                            nc.vector.tensor_tensor(
                                ob[:], stg[0:HD, :], rrep[:],
                                mybir.AluOpType.mult)
                            nc.sync.dma_start(o_all[HD:P, pg, qsl], ob[:])

                def emit_vmms(u):
                    pg, qt, kp, accs, e_aps = u
                    nears = near_kbs(qt)
                    ope, opo = accs
                    first_p = kp[0] == nears[0]
                    last_p = kp[1] == nears[-1]
                    for x in range(2):
                        for j, pkb in enumerate(kp):
                            jsl = slice(j * NQ, (j + 1) * NQ)
                            first = first_p and j == 0
                            last = last_p and j == 1
                            nc.tensor.matmul(
                                ope[x][0:HD + 1, :],
                                v_sb[0:HD, pkb, 2 * pg + x, :],
                                e_aps[x][0:HD, jsl],
                                start=first, stop=last,
                            )
                            nc.tensor.matmul(
                                opo[x][0:HD + 1, :],
                                v_sb[HD:P, pkb, 2 * pg + x, :],
                                e_aps[x][HD:P, jsl],
                                start=first, stop=last,
                            )
                    if last_p:
                        emit_norm(pg, qt, accs)

                units = []
                for pg in range(MPC):
                    for qt in range(NT):
                        nears = near_kbs(qt)
                        units.append((pg, qt,
                                      [(nears[2 * i], nears[2 * i + 1])
                                       for i in range(len(nears) // 2)]))

                pend = []
                accs = None
                for pg, qt, kpairs in units:
                    qsl = slice(qt * NQ, (qt + 1) * NQ)
                    accs = ([opool.tile([P, NQ], F32, tag="ops",
                                        name=f"ope{_n}") for _n in range(2)],
                            [opool.tile([P, NQ], F32, tag="ops",
                                        name=f"opo{_n}") for _n in range(2)])
                    for kp in kpairs:
                        pss = [spool.tile([P, 2 * NQ], F32, tag="spool",
                                          name=f"ps{_n}")
                               for _n in range(2)]
                        for j, kb in enumerate(kp):
                            cls = cls_of(qt, kb)
                            qvar, kvar = ((q_lo, k_lo) if cls != "hi"
                                          else (q_hi, k_hi))
                            ksl = slice(kb * P, (kb + 1) * P)
                            jsl = slice(j * NQ, (j + 1) * NQ)
                            for x in range(2):
                                psl = slice(x * HD, (x + 1) * HD)
                                nc.tensor.matmul(
                                    pss[x][:, jsl], kvar[psl, pg, ksl],
                                    qvar[psl, pg, qsl],
                                    start=True, stop=True,
                                )
                        e_aps = [None, None]
                        both_cross = all(cls_of(qt, kb) == "cross"
                                         for kb in kp)
                        for x in range(2):
                            if both_cross:
                                off = (kp[0] - 4 * qt) * NQ
                                nc.vector.tensor_tensor(
                                    pss[x][:], pss[x][:],
                                    corr_sb[:, off:off + 2 * NQ],
                                    mybir.AluOpType.mult)
                            elif any(cls_of(qt, kb) == "cross"
                                     for kb in kp):
                                for j, kb in enumerate(kp):
                                    if cls_of(qt, kb) != "cross":
                                        continue
                                    jsl = slice(j * NQ, (j + 1) * NQ)
                                    off = (kb - 4 * qt) * NQ
                                    nc.vector.tensor_tensor(
                                        pss[x][:, jsl], pss[x][:, jsl],
                                        corr_sb[:, off:off + NQ],
                                        mybir.AluOpType.mult)
                            e_t = epool.tile([P, 2 * NQ], BF16, tag="e")
                            nc.scalar.activation(
                                e_t[:], pss[x][:],
                                mybir.ActivationFunctionType.Exp)
                            e_aps[x] = e_t
                        if len(pend) >= 3:
                            emit_vmms(pend.pop(0))
                        for _d in range(2):
                            nc.tensor.ldweights(q_lo[:, 0, 0:P])
                        pend.append((pg, qt, kp, accs, e_aps))
                for u in pend:
                    emit_vmms(u)

            # ---- Stage C: output projection -------------------------------
            with tc.tile_pool(name="cpool", bufs=4, space="PSUM") as cpool, \
                 tc.tile_pool(name="fpool", bufs=4) as fpool:
                for mt in range(D // P):
                    for half in range(2):
                        hs = half * (T // 2)
                        pc = cpool.tile([P, T // 2], F32, tag="cps")
                        for kt2 in range(2):
                            for nt in range(2):
                                nc.tensor.matmul(
                                    pc[:, nt * NQ:(nt + 1) * NQ],
                                    wo_sb[:, kt2, mt * P:(mt + 1) * P],
                                    o_all[:, kt2,
                                          hs + nt * NQ:hs + (nt + 1) * NQ],
                                    start=(kt2 == 0), stop=(kt2 == 1),
                                )
                        fo = fpool.tile([P, T // 2], BF16, tag="fo")
                        if (2 * mt + half) % 2 == 0:
                            nc.vector.tensor_copy(fo[:], pc[:])
                        else:
                            nc.scalar.copy(fo[:], pc[:])
                        nc.sync.dma_start(
                            out_ext[mt * P:(mt + 1) * P, hs:hs + T // 2],
                            fo[:])

    nc.finalize()
    return nc


def _get_graph(kp):
    if kp not in _GRAPH_CACHE:
        _GRAPH_CACHE[kp] = _build(kp)
    return _GRAPH_CACHE[kp]


def _install_trace_hooks():
    import types
    import antenv
    if "antenv.axon_hooks" not in sys.modules:
        hooks = types.ModuleType("antenv.axon_hooks")
        hooks._hook = None
        hooks.set_axon_ntff_profile_hook = lambda h: setattr(hooks, "_hook", h)
        hooks.get_axon_ntff_profile_hook = lambda: hooks._hook
        sys.modules["antenv.axon_hooks"] = hooks
        antenv.axon_hooks = hooks
    if sys.modules["antenv.axon_hooks"]._hook is None:
        if "/root/.axon_site" not in sys.path:
            sys.path.insert(0, "/root/.axon_site")
        from trn_agent_boot.trn_boot import _ntff_profile_via_ctypes
        sys.modules["antenv.axon_hooks"].set_axon_ntff_profile_hook(
            _ntff_profile_via_ctypes("/opt/axon/libaxon_pjrt.so"))


def kernel(x, Wq, bq, Wk, bk, Wv, bv, Wo, bo, alpha):
    global LAST_EXEC_NS, LAST_RESULT
    x = np.asarray(x, dtype=np.float32)
    Wq = np.asarray(Wq, dtype=np.float32)
    Wk = np.asarray(Wk, dtype=np.float32)
    Wv = np.asarray(Wv, dtype=np.float32)
    Wo = np.asarray(Wo, dtype=np.float32)
    bq = np.asarray(bq, dtype=np.float32)
    bk = np.asarray(bk, dtype=np.float32)
    bv = np.asarray(bv, dtype=np.float32)
    bo = np.asarray(bo, dtype=np.float32)
    alpha = float(np.asarray(alpha))
    a_eff = alpha * DT_CONST
    scale = HD ** -0.5

    has_bias = bool(np.any(bq) or np.any(bk) or np.any(bv))
    kp = 9 if has_bias else 8
    nc = _get_graph(kp)

    t_idx = np.arange(T, dtype=np.float64)
    e_neg = np.exp(-a_eff * t_idx)
    e_pos = np.exp(+a_eff * t_idx)
    rqlo = np.tile((scale * e_neg).astype(np.float32), (P, 1))
    rqhi = np.tile((scale * e_pos).astype(np.float32), (P, 1))
    rklo = np.tile(e_pos.astype(np.float32), (P, 1))
    rkhi = np.tile(e_neg.astype(np.float32), (P, 1))

    # corr[kk, o*512+qq] = 1 if d>=0 else exp(2*a_eff*d), d = qq-kk-128*o
    kk = np.arange(P)[:, None]
    qq = np.arange(NQ)[None, :]
    corr = np.empty((P, T), dtype=np.float32)
    for o in range(4):
        d = qq - kk - P * o
        corr[:, o * NQ:(o + 1) * NQ] = np.where(
            d >= 0, 1.0, np.exp(2.0 * a_eff * d))

    def wslice(W, b, g):
        ws = W[256 * g:256 * g + 256, :].T.astype(np.float64)
        if has_bias:
            ws = np.vstack([ws, b[256 * g:256 * g + 256][None, :],
                            np.zeros((kp * P - D - 1, 256))])
        return np.ascontiguousarray(ws).astype(ml_dtypes.bfloat16)

    in_maps = []
    for core in range(NCORES):
        b_idx, g = core // 4, core % 4
        xT = x[b_idx].T.astype(np.float64)
        if has_bias:
            xT = np.vstack([xT, np.ones((1, T)), np.zeros((kp * P - D - 1, T))])
        in_maps.append({
            "xT": np.ascontiguousarray(xT).astype(ml_dtypes.bfloat16),
            "wq": wslice(Wq, bq, g),
            "wk": wslice(Wk, bk, g),
            "wv": wslice(Wv, bv, g),
            "wo": np.ascontiguousarray(
                Wo[:, 256 * g:256 * g + 256].T).astype(ml_dtypes.bfloat16),
            "rqlo": rqlo, "rqhi": rqhi, "rklo": rklo, "rkhi": rkhi,
            "corr": corr,
        })

    trace = bool(os.environ.get("BASS_KERNEL_TRACE"))
    if trace:
        _install_trace_hooks()
    res = run_bass_kernel_spmd(nc, in_maps, core_ids=list(range(NCORES)),
                               trace=trace)
    LAST_EXEC_NS = res.exec_time_ns
    LAST_RESULT = res

    out = np.empty((B, T, D), dtype=np.float32)
    for b_idx in range(B):
        acc = np.zeros((D, T), dtype=np.float32)
        for g in range(4):
            acc += np.asarray(res.results[b_idx * 4 + g]["out"],
                              dtype=np.float32)
        out[b_idx] = acc.T + bo[None, :]
    return out
